# revision 1
# baseline (speedup 1.0000x reference)
"""Trainium2 Bass kernel for nn_Attention_72670846649042.

GRU encoder + greedy attention decoder, B=512,L=25,H=1024,D=256,T=128,E=300.
Sharding: data-parallel over batch, 64 rows/core on 8 cores, no collectives.
Compute dtype bf16 (validated host-side: rel_err ~2.4e-3 vs fp32 reference).

Layouts per core (b = local batch 0..63):
 - state h kept twice: h fp32 (64p, 1024f) for elementwise; hT bf16 (128p, 8*64f)
   as matmul lhsT (ktile k at cols [k*64,(k+1)*64)).
 - weights pre-transposed on host, streamed as matmul rhs in bf16.
 - encoder input proj gi_all bounced through DRAM (SBUF can't hold it + weights).
 - attention einsum via PSUM-accumulated block-diag matmuls (2 l-steps/pair).
"""
import os
import numpy as np
import ml_dtypes

B, L, V, E, H, D, T = 512, 25, 50000, 300, 1024, 256, 128
NC = 8
BL = B // NC          # 64 local batch
G3 = 3 * H            # 3072
KH = H // 128         # 8 hidden ktiles
NCH = G3 // 512       # 6 gate n-chunks
MAXN1, MAXN2, BN_EPS = 10.0, 1.0, 1e-5
MT = 13               # xT M-tiles (1664 = 13*128 >= 1600)
BF16 = ml_dtypes.bfloat16


LINEARIZE = False


def build_nc():
    import concourse.bass as bass
    import concourse.tile as tile
    from concourse import bacc, mybir
    from contextlib import ExitStack

    dt = mybir.dt
    AF = mybir.ActivationFunctionType
    ALU = mybir.AluOpType
    AX = mybir.AxisListType

    nc = bacc.Bacc("TRN2", target_bir_lowering=False, debug=False)

    # ---- dram parameters (per-core shards / replicated weights) ----
    xT_d = nc.declare_dram_parameter("xT", [E, MT * 128], dt.float32, isOutput=False)
    encWihT_d = nc.declare_dram_parameter("encWihT", [E, G3], dt.bfloat16, isOutput=False)
    encWhhT_d = nc.declare_dram_parameter("encWhhT", [H, G3], dt.bfloat16, isOutput=False)
    decWihT_d = nc.declare_dram_parameter("decWihT", [H, G3], dt.bfloat16, isOutput=False)
    decWhhT_d = nc.declare_dram_parameter("decWhhT", [H, G3], dt.bfloat16, isOutput=False)
    combWT_d = nc.declare_dram_parameter("combWT", [D + H, H], dt.bfloat16, isOutput=False)
    outWTs_d = nc.declare_dram_parameter("outWTs", [H, T], dt.bfloat16, isOutput=False)
    attnWT_d = nc.declare_dram_parameter("attnWT", [D + H, L], dt.bfloat16, isOutput=False)
    embm_d = nc.declare_dram_parameter("embm", [128, D], dt.float32, isOutput=False)
    sosr_d = nc.declare_dram_parameter("sosr", [BL, D], dt.float32, isOutput=False)
    # broadcast-ready bias rows (1, X): replicated on device
    egib_d = nc.declare_dram_parameter("egib", [1, G3], dt.bfloat16, isOutput=False)
    ebhn_d = nc.declare_dram_parameter("ebhn", [1, H], dt.bfloat16, isOutput=False)
    dgib_d = nc.declare_dram_parameter("dgib", [1, G3], dt.bfloat16, isOutput=False)
    dbhn_d = nc.declare_dram_parameter("dbhn", [1, H], dt.bfloat16, isOutput=False)
    combb_d = nc.declare_dram_parameter("combb", [1, H], dt.bfloat16, isOutput=False)
    attnb_d = nc.declare_dram_parameter("attnb", [1, L], dt.bfloat16, isOutput=False)
    lgb_d = nc.declare_dram_parameter("lgb", [1, T], dt.bfloat16, isOutput=False)
    istk_d = nc.declare_dram_parameter("istk", [128, BL], dt.bfloat16, isOutput=False)
    out_d = nc.declare_dram_parameter("out", [BL * L, T], dt.float32, isOutput=True)

    gi_dram = nc.dram_tensor("gi_bounce", [MT * 128, G3], dt.float32, kind="Internal")

    with tile.TileContext(nc, linearize=LINEARIZE) as tc, ExitStack() as ctx:
        # ---------- persistent pools ----------
        shared = ctx.enter_context(tc.tile_pool(name="shared", bufs=1))
        pre_cm = tc.tile_pool(name="pre", bufs=1)
        pre = pre_cm.__enter__()

        attnWT = shared.tile([128, (D + H) // 128, L], dt.bfloat16, tag="attnWT")
        nc.sync.dma_start(attnWT[:], attnWT_d.ap().rearrange("(k p) n -> p k n", p=128))

        enc_out = shared.tile([128, MT, H], dt.bfloat16, tag="enc_out")
        hT = shared.tile([128, KH * BL], dt.bfloat16, tag="hT")
        h_cur = shared.tile([BL, H], dt.float32, tag="h_cur")
        embT = shared.tile([128, 2 * BL], dt.bfloat16, tag="embT")
        emb_bf = shared.tile([128, D], dt.bfloat16, tag="emb_bf")
        Istk = shared.tile([128, BL], dt.bfloat16, tag="Istk")
        ones_sb = shared.tile([1, 128], dt.bfloat16, tag="ones_sb")

        # bias rows (1, X) accumulated into PSUM via K=1 ones-matmuls
        dgib_r = shared.tile([1, G3], dt.bfloat16, tag="dgib_r")
        nc.sync.dma_start(dgib_r[:], dgib_d.ap())
        ebhn_r = shared.tile([1, H], dt.bfloat16, tag="ebhn_r")
        nc.sync.dma_start(ebhn_r[:], ebhn_d.ap())
        dbhn_r = shared.tile([1, H], dt.bfloat16, tag="dbhn_r")
        nc.sync.dma_start(dbhn_r[:], dbhn_d.ap())
        combb_r = shared.tile([1, H], dt.bfloat16, tag="combb_r")
        nc.sync.dma_start(combb_r[:], combb_d.ap())
        attnb_r = shared.tile([1, L], dt.bfloat16, tag="attnb_r")
        nc.sync.dma_start(attnb_r[:], attnb_d.ap())
        lgb_r = shared.tile([1, T], dt.bfloat16, tag="lgb_r")
        nc.sync.dma_start(lgb_r[:], lgb_d.ap())
        nc.vector.memset(ones_sb[:], 1.0)

        # I_stack[p, j] = (p % 64 == j), bf16 (constant, shipped from host)
        nc.sync.dma_start(Istk[:], istk_d.ap())

        # ---- dec_emb renorm (rows 0..127) -> emb_bf (lhsT for embT matmul) ----
        embm = pre.tile([128, D], dt.float32, tag="embm")
        nc.sync.dma_start(embm[:], embm_d.ap())
        sq = pre.tile([128, D], dt.float32, tag="sq")
        nc.vector.tensor_tensor(sq[:], embm[:], embm[:], op=ALU.mult)
        ssum = pre.tile([128, 1], dt.float32, tag="ssum")
        nc.vector.tensor_reduce(ssum[:], sq[:], axis=AX.X, op=ALU.add)
        nrm = pre.tile([128, 1], dt.float32, tag="nrm")
        nc.scalar.activation(nrm[:], ssum[:], AF.Sqrt)
        nc.vector.tensor_scalar(nrm[:], nrm[:], 1e-7, None, op0=ALU.add)
        rcp = pre.tile([128, 1], dt.float32, tag="rcp")
        nc.vector.reciprocal(rcp[:], nrm[:])
        scl = pre.tile([128, 1], dt.float32, tag="scl")
        nc.vector.tensor_scalar(scl[:], rcp[:], MAXN2, 1.0, op0=ALU.mult, op1=ALU.min)
        nc.vector.tensor_scalar(emb_bf[:], embm[:], scl[:], None, op0=ALU.mult)

        # ---- SOS embedding (replicated rows) -> embT for step 0 ----
        sos = pre.tile([BL, D], dt.float32, tag="sos")
        nc.sync.dma_start(sos[:], sosr_d.ap())
        sq2 = pre.tile([BL, D], dt.float32, tag="sq2")
        nc.vector.tensor_tensor(sq2[:], sos[:], sos[:], op=ALU.mult)
        ssum2 = pre.tile([BL, 1], dt.float32, tag="ssum2")
        nc.vector.tensor_reduce(ssum2[:], sq2[:], axis=AX.X, op=ALU.add)
        nrm2 = pre.tile([BL, 1], dt.float32, tag="nrm2")
        nc.scalar.activation(nrm2[:], ssum2[:], AF.Sqrt)
        nc.vector.tensor_scalar(nrm2[:], nrm2[:], 1e-7, None, op0=ALU.add)
        rcp2 = pre.tile([BL, 1], dt.float32, tag="rcp2")
        nc.vector.reciprocal(rcp2[:], nrm2[:])
        scl2 = pre.tile([BL, 1], dt.float32, tag="scl2")
        nc.vector.tensor_scalar(scl2[:], rcp2[:], MAXN2, 1.0, op0=ALU.mult, op1=ALU.min)
        sos_bf = pre.tile([BL, D], dt.bfloat16, tag="sos_bf")
        nc.vector.tensor_scalar(sos_bf[:], sos[:], scl2[:], None, op0=ALU.mult)
        for k in range(2):
            nc.sync.dma_start_transpose(embT[:, k * BL:(k + 1) * BL],
                                        sos_bf[:, k * 128:(k + 1) * 128])

        # init h = 0, hT = 0; zero pad half of last enc_out pair
        nc.vector.memset(h_cur[:], 0.0)
        nc.vector.memset(hT[:], 0.0)
        nc.vector.memset(enc_out[BL:128, MT - 1, :], 0.0)

        pre_cm.__exit__(None, None, None)

        # =======================================================
        # Phase 1: encoder input projection -> gi_dram (bf16)
        # gi' = renorm(x) @ encWihT + (bih + bhh_rz-folded)
        # =======================================================
        with tc.tile_pool(name="proj", bufs=2) as projp, \
             tc.tile_pool(name="projps", bufs=6, space="PSUM") as projps, \
             tc.tile_pool(name="npsp", bufs=1, space="PSUM") as npsp, \
             tc.tile_pool(name="wih", bufs=1) as wihp:
            onesk = wihp.tile([128, 1], dt.bfloat16, tag="onesk")
            nc.vector.memset(onesk[:], 1.0)
            egib_r = wihp.tile([1, G3], dt.bfloat16, tag="egib_r")
            nc.sync.dma_start(egib_r[:], egib_d.ap())
            encWihT = wihp.tile([128, 3, G3], dt.bfloat16, tag="encWihT")
            # E=300 ktiles: 128,128,44
            nc.sync.dma_start(encWihT[:, 0, :], encWihT_d.ap()[0:128, :])
            nc.sync.dma_start(encWihT[:, 1, :], encWihT_d.ap()[128:256, :])
            nc.sync.dma_start(encWihT[0:44, 2, :], encWihT_d.ap()[256:300, :])
            for m in range(MT):
                xt = projp.tile([128, 3, 128], dt.float32, tag="xt")
                nc.sync.dma_start(xt[:, 0, :], xT_d.ap()[0:128, m * 128:(m + 1) * 128])
                nc.sync.dma_start(xt[:, 1, :], xT_d.ap()[128:256, m * 128:(m + 1) * 128])
                nc.sync.dma_start(xt[0:44, 2, :], xT_d.ap()[256:300, m * 128:(m + 1) * 128])
                # col norms via ones-matmul over squared tiles
                xsq = projp.tile([128, 3, 128], dt.bfloat16, tag="xsq")
                kr = (128, 128, 44)
                for k in range(3):
                    nc.vector.tensor_tensor(xsq[0:kr[k], k, :], xt[0:kr[k], k, :],
                                            xt[0:kr[k], k, :], op=ALU.mult)
                nps = npsp.tile([1, 512], dt.float32, tag="nps")
                for k in range(3):
                    nc.tensor.matmul(nps[0:1, 0:128], onesk[0:kr[k], :],
                                     xsq[0:kr[k], k, :], start=(k == 0), stop=(k == 2))
                nrm3 = projp.tile([1, 128], dt.float32, tag="nrm3")
                nc.scalar.activation(nrm3[:], nps[0:1, 0:128], AF.Sqrt)
                nc.vector.tensor_scalar(nrm3[:], nrm3[:], 1e-7, None, op0=ALU.add)
                rcp3 = projp.tile([1, 128], dt.float32, tag="rcp3")
                nc.vector.reciprocal(rcp3[:], nrm3[:])
                nc.vector.tensor_scalar(rcp3[:], rcp3[:], MAXN1, 1.0, op0=ALU.mult, op1=ALU.min)
                sclb = projp.tile([128, 128], dt.float32, tag="sclb")
                rcp3b = projp.tile([1, 128], dt.bfloat16, tag="rcp3b")
                nc.vector.tensor_copy(rcp3b[:], rcp3[:])
                sps = npsp.tile([128, 512], dt.float32, tag="sps")
                nc.tensor.matmul(sps[:, 0:128], ones_sb[0:1, :], rcp3b[:],
                                 start=True, stop=True)
                nc.vector.tensor_copy(sclb[:], sps[:, 0:128])
                xbf = projp.tile([128, 3, 128], dt.bfloat16, tag="xbf")
                for k in range(3):
                    nc.vector.tensor_tensor(xbf[0:kr[k], k, :], xt[0:kr[k], k, :],
                                            sclb[0:kr[k], :], op=ALU.mult)
                for n in range(NCH):
                    gps = projps.tile([128, 512], dt.float32, tag="gps")
                    for k in range(3):
                        nc.tensor.matmul(gps[:], xbf[0:kr[k], k, :],
                                         encWihT[0:kr[k], k, n * 512:(n + 1) * 512],
                                         start=(k == 0), stop=False)
                    nc.tensor.matmul(gps[:], ones_sb[0:1, :],
                                     egib_r[0:1, n * 512:(n + 1) * 512],
                                     start=False, stop=True)
                    gsb = projp.tile([128, 512], dt.float32, tag="gsb")
                    nc.scalar.copy(gsb[:], gps[:])
                    nc.sync.dma_start(gi_dram.ap()[m * 128:(m + 1) * 128,
                                                   n * 512:(n + 1) * 512], gsb[:])

        # =======================================================
        # Phase 2: encoder GRU scan (25 steps)
        # =======================================================
        def gru_step(gi_r, gi_z, gi_n_plus, ps_r, ps_z, ps_n, hh, work, dst_bf=None,
                     dst_bf_sl=None):
            """gates fp32 (biases pre-accumulated in psum): r=sig(ps_r+gi_r)
            z=sig(ps_z+gi_z) n=tanh(gi_n_plus + r*ps_n); h2=n+z*(hh-n)"""
            r_s = work.tile([BL, 512], dt.float32, tag="r_s")
            nc.vector.tensor_tensor(r_s[:], ps_r, gi_r, op=ALU.add)
            nc.scalar.activation(r_s[:], r_s[:], AF.Sigmoid)
            z_s = work.tile([BL, 512], dt.float32, tag="z_s")
            nc.vector.tensor_tensor(z_s[:], ps_z, gi_z, op=ALU.add)
            nc.scalar.activation(z_s[:], z_s[:], AF.Sigmoid)
            n_s = work.tile([BL, 512], dt.float32, tag="n_s")
            nc.vector.tensor_tensor(n_s[:], ps_n, r_s[:], op=ALU.mult)
            nc.vector.tensor_tensor(n_s[:], n_s[:], gi_n_plus, op=ALU.add)
            nc.scalar.activation(n_s[:], n_s[:], AF.Tanh)
            t4 = work.tile([BL, 512], dt.float32, tag="t4")
            nc.vector.tensor_tensor(t4[:], hh, n_s[:], op=ALU.subtract)
            nc.vector.tensor_tensor(t4[:], t4[:], z_s[:], op=ALU.mult)
            nc.vector.tensor_tensor(hh, n_s[:], t4[:], op=ALU.add)
            if dst_bf is not None:
                nc.scalar.copy(dst_bf_sl, hh)

        with tc.tile_pool(name="enc", bufs=2) as encp, \
             tc.tile_pool(name="encw", bufs=1) as encwp, \
             tc.tile_pool(name="encps", bufs=2, space="PSUM") as encps, \
             tc.tile_pool(name="work", bufs=2) as work:
            encWhhT = encwp.tile([128, KH, G3], dt.bfloat16, tag="encWhhT")
            nc.sync.dma_start(encWhhT[:], encWhhT_d.ap().rearrange("(k p) n -> p k n", p=128))
            for t in range(L):
                gi = encp.tile([BL, G3], dt.float32, tag="gi")
                nc.sync.dma_start(gi[:], gi_dram.ap()[t * 64:(t + 1) * 64, :])
                h2bf = encp.tile([BL, H], dt.bfloat16, tag="h2bf")
                for hc in range(2):
                    ps = [encps.tile([BL, 512], dt.float32, name=f"g{g}", tag=f"g{g}") for g in range(3)]
                    for g in range(3):
                        nco = g * H + hc * 512
                        for k in range(KH):
                            nc.tensor.matmul(ps[g][:], hT[:, k * BL:(k + 1) * BL],
                                             encWhhT[:, k, nco:nco + 512],
                                             start=(k == 0),
                                             stop=(k == KH - 1 and g != 2))
                    nc.tensor.matmul(ps[2][:], ones_sb[0:1, 0:BL],
                                     ebhn_r[0:1, hc * 512:hc * 512 + 512],
                                     start=False, stop=True)
                    sl = slice(hc * 512, hc * 512 + 512)
                    gru_step(gi[:, 0 * H + hc * 512:0 * H + hc * 512 + 512],
                             gi[:, 1 * H + hc * 512:1 * H + hc * 512 + 512],
                             gi[:, 2 * H + hc * 512:2 * H + hc * 512 + 512],
                             ps[0][:], ps[1][:], ps[2][:], h_cur[:, sl],
                             work, dst_bf=h2bf, dst_bf_sl=h2bf[:, sl])
                # store enc_out pair slice + refresh hT
                po = (t % 2) * BL
                nc.vector.tensor_copy(enc_out[po:po + BL, t // 2, :], h2bf[:])
                for k in range(KH):
                    nc.sync.dma_start_transpose(hT[:, k * BL:(k + 1) * BL],
                                                h2bf[:, k * 128:(k + 1) * 128])

        # =======================================================
        # Phase 3: decoder (25 steps)
        # =======================================================
        with tc.tile_pool(name="decw", bufs=1) as decwp, \
             tc.tile_pool(name="dec", bufs=2) as decp, \
             tc.tile_pool(name="decps", bufs=8, space="PSUM") as decps, \
             tc.tile_pool(name="work2", bufs=2) as work2:
            decWhhT = decwp.tile([128, KH, G3], dt.bfloat16, tag="decWhhT")
            nc.sync.dma_start(decWhhT[:], decWhhT_d.ap().rearrange("(k p) n -> p k n", p=128))
            decWihT = decwp.tile([128, KH, G3], dt.bfloat16, tag="decWihT")
            nc.sync.dma_start(decWihT[:], decWihT_d.ap().rearrange("(k p) n -> p k n", p=128))
            outWTs = decwp.tile([128, KH, T], dt.bfloat16, tag="outWTs")
            nc.sync.dma_start(outWTs[:], outWTs_d.ap().rearrange("(k p) n -> p k n", p=128))
            combWT = decwp.tile([128, 10, H], dt.bfloat16, tag="combWT")
            nc.sync.dma_start(combWT[:], combWT_d.ap().rearrange("(k p) n -> p k n", p=128))
            for t in range(L):
                # ---- attention scores (64,25): lhsT = [embT(2); hT(8)] ----
                scps = decps.tile([BL, 512], dt.float32, tag="ps")
                for k in range(10):
                    lhs = embT[:, (k) * BL:(k + 1) * BL] if k < 2 else \
                        hT[:, (k - 2) * BL:(k - 1) * BL]
                    nc.tensor.matmul(scps[:, 0:L], lhs, attnWT[:, k, :],
                                     start=(k == 0), stop=False)
                nc.tensor.matmul(scps[:, 0:L], ones_sb[0:1, 0:BL], attnb_r[:],
                                 start=False, stop=True)
                mx = decp.tile([BL, 1], dt.float32, tag="mx")
                nc.vector.tensor_reduce(mx[:], scps[:, 0:L], axis=AX.X, op=ALU.max)
                nmx = decp.tile([BL, 1], dt.float32, tag="nmx")
                nc.vector.tensor_scalar(nmx[:], mx[:], -1.0, None, op0=ALU.mult)
                aw = decp.tile([BL, L], dt.float32, tag="aw")
                sume = decp.tile([BL, 1], dt.float32, tag="sume")
                nc.scalar.activation(aw[:], scps[:, 0:L], AF.Exp, bias=nmx[:],
                                     accum_out=sume[:])
                rs = decp.tile([BL, 1], dt.float32, tag="rs")
                nc.vector.reciprocal(rs[:], sume[:])
                # aw_shift (128, L): top=aw, bottom=aw shifted left by 1 (pad 0)
                awsh = decp.tile([128, L], dt.float32, tag="awsh")
                nc.vector.memset(awsh[BL:128, L - 1:L], 0.0)
                nc.vector.tensor_copy(awsh[0:BL, :], aw[:])
                nc.vector.tensor_copy(awsh[BL:128, 0:L - 1], aw[:, 1:L])
                rs2 = decp.tile([128, 1], dt.float32, tag="rs2")
                nc.vector.tensor_copy(rs2[0:BL, :], rs[:])
                nc.vector.tensor_copy(rs2[BL:128, :], rs[:])
                # ---- applied (64,1024) = sum_l aw[b,l] enc_out[b,l,:] ----
                dgs = decp.tile([128, MT * BL], dt.bfloat16, tag="dgs", bufs=1)
                for p in range(MT):
                    nc.vector.tensor_scalar(dgs[:, p * BL:(p + 1) * BL], Istk[:],
                                            awsh[:, 2 * p:2 * p + 1], rs2[:],
                                            op0=ALU.mult, op1=ALU.mult)
                aps0 = decps.tile([BL, 512], dt.float32, tag="ps")
                aps1 = decps.tile([BL, 512], dt.float32, tag="ps")
                for p in range(MT):
                    nc.tensor.matmul(aps0[:], dgs[:, p * BL:(p + 1) * BL],
                                     enc_out[:, p, 0:512], start=(p == 0), stop=(p == MT - 1))
                    nc.tensor.matmul(aps1[:], dgs[:, p * BL:(p + 1) * BL],
                                     enc_out[:, p, 512:1024], start=(p == 0), stop=(p == MT - 1))
                apbf = decp.tile([BL, H], dt.bfloat16, tag="apbf")
                nc.scalar.copy(apbf[:, 0:512], aps0[:])
                nc.scalar.copy(apbf[:, 512:1024], aps1[:])
                apT = decp.tile([128, KH * BL], dt.bfloat16, tag="apT")
                for k in range(KH):
                    nc.sync.dma_start_transpose(apT[:, k * BL:(k + 1) * BL],
                                                apbf[:, k * 128:(k + 1) * 128])
                # ---- comb + bn2 + relu: o = relu(s2*(mm + combb')) ----
                obf = decp.tile([BL, H], dt.bfloat16, tag="obf")
                for n in range(2):
                    ops = decps.tile([BL, 512], dt.float32, tag="ps")
                    for k in range(10):
                        lhs = embT[:, k * BL:(k + 1) * BL] if k < 2 else \
                            apT[:, (k - 2) * BL:(k - 1) * BL]
                        nc.tensor.matmul(ops[:], lhs, combWT[:, k, n * 512:(n + 1) * 512],
                                         start=(k == 0), stop=False)
                    nc.tensor.matmul(ops[:], ones_sb[0:1, 0:BL],
                                     combb_r[0:1, n * 512:(n + 1) * 512],
                                     start=False, stop=True)
                    nc.scalar.activation(obf[:, n * 512:(n + 1) * 512], ops[:], AF.Relu,
                                         scale=S2_SCALE)
                oT = decp.tile([128, KH * BL], dt.bfloat16, tag="oT")
                for k in range(KH):
                    nc.sync.dma_start_transpose(oT[:, k * BL:(k + 1) * BL],
                                                obf[:, k * 128:(k + 1) * 128])
                # ---- GRU: gh from hT@decWhhT, gi from oT@decWihT ----
                h2bf = decp.tile([BL, H], dt.bfloat16, tag="h2bf2")
                for hc in range(2):
                    # r,z gates: gh+gi+bias all accumulated into ONE psum each
                    prz = [decps.tile([BL, 512], dt.float32, name=f"prz{g}", tag="ps")
                           for g in range(2)]
                    for g in range(2):
                        nco = g * H + hc * 512
                        for k in range(KH):
                            nc.tensor.matmul(prz[g][:], hT[:, k * BL:(k + 1) * BL],
                                             decWhhT[:, k, nco:nco + 512],
                                             start=(k == 0), stop=False)
                        for k in range(KH):
                            nc.tensor.matmul(prz[g][:], oT[:, k * BL:(k + 1) * BL],
                                             decWihT[:, k, nco:nco + 512],
                                             start=False, stop=False)
                        nc.tensor.matmul(prz[g][:], ones_sb[0:1, 0:BL],
                                         dgib_r[0:1, nco:nco + 512],
                                         start=False, stop=True)
                    # n gate: gh_n+bhh_n and gi_n+bih_n kept separate
                    nco = 2 * H + hc * 512
                    pgn = decps.tile([BL, 512], dt.float32, name="pgn", tag="ps")
                    for k in range(KH):
                        nc.tensor.matmul(pgn[:], hT[:, k * BL:(k + 1) * BL],
                                         decWhhT[:, k, nco:nco + 512],
                                         start=(k == 0), stop=False)
                    nc.tensor.matmul(pgn[:], ones_sb[0:1, 0:BL],
                                     dbhn_r[0:1, hc * 512:hc * 512 + 512],
                                     start=False, stop=True)
                    pin = decps.tile([BL, 512], dt.float32, name="pin", tag="ps")
                    for k in range(KH):
                        nc.tensor.matmul(pin[:], oT[:, k * BL:(k + 1) * BL],
                                         decWihT[:, k, nco:nco + 512],
                                         start=(k == 0), stop=False)
                    nc.tensor.matmul(pin[:], ones_sb[0:1, 0:BL],
                                     dgib_r[0:1, nco:nco + 512],
                                     start=False, stop=True)
                    sl = slice(hc * 512, hc * 512 + 512)
                    r_s = work2.tile([BL, 512], dt.float32, tag="r_s")
                    nc.scalar.activation(r_s[:], prz[0][:], AF.Sigmoid)
                    z_s = work2.tile([BL, 512], dt.float32, tag="z_s")
                    nc.scalar.activation(z_s[:], prz[1][:], AF.Sigmoid)
                    n_s = work2.tile([BL, 512], dt.float32, tag="n_s")
                    nc.vector.tensor_tensor(n_s[:], pgn[:], r_s[:], op=ALU.mult)
                    nc.vector.tensor_tensor(n_s[:], n_s[:], pin[:], op=ALU.add)
                    nc.scalar.activation(n_s[:], n_s[:], AF.Tanh)
                    t4 = work2.tile([BL, 512], dt.float32, tag="t4")
                    nc.vector.tensor_tensor(t4[:], h_cur[:, sl], n_s[:], op=ALU.subtract)
                    nc.vector.tensor_tensor(t4[:], t4[:], z_s[:], op=ALU.mult)
                    nc.vector.tensor_tensor(h_cur[:, sl], n_s[:], t4[:], op=ALU.add)
                    nc.scalar.copy(h2bf[:, sl], h_cur[:, sl])
                for k in range(KH):
                    nc.sync.dma_start_transpose(hT[:, k * BL:(k + 1) * BL],
                                                h2bf[:, k * 128:(k + 1) * 128])
                # ---- logits (64,128) = h2T @ outWTs + lgb ----
                lps = decps.tile([BL, 512], dt.float32, tag="ps")
                for k in range(KH):
                    nc.tensor.matmul(lps[:, 0:T], hT[:, k * BL:(k + 1) * BL],
                                     outWTs[:, k, :], start=(k == 0), stop=False)
                nc.tensor.matmul(lps[:, 0:T], ones_sb[0:1, 0:BL], lgb_r[:],
                                 start=False, stop=True)
                lg = decp.tile([BL, T], dt.float32, tag="lg")
                nc.vector.tensor_copy(lg[:], lps[:, 0:T])
                # ---- argmax -> onehot -> next embT (skip at last step) ----
                mx2 = decp.tile([BL, 1], dt.float32, tag="mx2")
                nc.vector.tensor_reduce(mx2[:], lg[:], axis=AX.X, op=ALU.max)
                if t < L - 1:
                    oh = decp.tile([BL, T], dt.bfloat16, tag="oh")
                    nc.vector.tensor_scalar(oh[:], lg[:], mx2[:], None, op0=ALU.is_equal)
                    ohT = decp.tile([128, BL], dt.bfloat16, tag="ohT")
                    nc.sync.dma_start_transpose(ohT[:], oh[:])
                    for k in range(2):
                        eps = decps.tile([128, 512], dt.float32, tag="ps")
                        nc.tensor.matmul(eps[:, 0:BL], emb_bf[:, k * 128:(k + 1) * 128],
                                         ohT[:], start=True, stop=True)
                        nc.vector.tensor_copy(embT[:, k * BL:(k + 1) * BL], eps[:, 0:BL])
                # ---- log_softmax -> logits_all ----
                nmx2 = decp.tile([BL, 1], dt.float32, tag="nmx2")
                nc.vector.tensor_scalar(nmx2[:], mx2[:], -1.0, None, op0=ALU.mult)
                ex = decp.tile([BL, T], dt.float32, tag="ex")
                se = decp.tile([BL, 1], dt.float32, tag="se")
                nc.scalar.activation(ex[:], lg[:], AF.Exp, bias=nmx2[:], accum_out=se[:])
                lse = decp.tile([BL, 1], dt.float32, tag="lse")
                nc.scalar.activation(lse[:], se[:], AF.Ln)
                nc.vector.tensor_tensor(lse[:], lse[:], mx2[:], op=ALU.add)
                lout = decp.tile([BL, T], dt.float32, tag="lout")
                nc.vector.tensor_scalar(lout[:], lg[:], lse[:], None, op0=ALU.subtract)
                nc.sync.dma_start(
                    out_d.ap().rearrange("(b l) c -> b l c", l=L)[:, t, :], lout[:])
    nc.finalize()
    return nc


S2_SCALE = 1.0  # patched at build time (bn2 scale); module-level for closure use


def kernel(**inputs):
    global S2_SCALE
    import concourse.bass_utils as bass_utils

    tokens = np.asarray(inputs["tokens"])
    tok_dtype = tokens.dtype
    w2v = np.asarray(inputs["w2v"], np.float32)
    bn1 = np.asarray(inputs["bn1"], np.float32)
    bn2 = np.asarray(inputs["bn2"], np.float32)
    s1 = float(bn1[0] / np.sqrt(bn1[3] + BN_EPS))
    t1 = float(bn1[1] - bn1[2] * s1)
    s2 = float(bn2[0] / np.sqrt(bn2[3] + BN_EPS))
    t2 = float(bn2[1] - bn2[2] * s2)
    S2_SCALE = s2

    f32 = lambda k: np.asarray(inputs[k], np.float32)
    bft = lambda a: np.ascontiguousarray(np.asarray(a, np.float32).T).astype(BF16)
    enc_bih, enc_bhh = f32("enc_bih"), f32("enc_bhh")
    dec_bih, dec_bhh = f32("dec_bih"), f32("dec_bhh")
    egib = np.concatenate([enc_bih[:H] + enc_bhh[:H], enc_bih[H:2 * H] + enc_bhh[H:2 * H],
                           enc_bih[2 * H:]])[None, :]
    dgib = np.concatenate([dec_bih[:H] + dec_bhh[:H], dec_bih[H:2 * H] + dec_bhh[H:2 * H],
                           dec_bih[2 * H:]])[None, :]
    out_W = f32("out_W")
    outWTs = np.ascontiguousarray((s1 * out_W).T).astype(BF16)
    lgb = (f32("out_b") + t1 * out_W.sum(axis=1))[None, :]
    combb = (f32("comb_b") + t2 / s2)[None, :]

    common = {
        "encWihT": bft(inputs["enc_Wih"]), "encWhhT": bft(inputs["enc_Whh"]),
        "decWihT": bft(inputs["dec_Wih"]), "decWhhT": bft(inputs["dec_Whh"]),
        "combWT": bft(inputs["comb_W"]), "outWTs": outWTs,
        "attnWT": bft(inputs["attn_W"]),
        "embm": np.asarray(inputs["dec_emb"][:128], np.float32),
        "sosr": np.ascontiguousarray(
            np.broadcast_to(np.asarray(inputs["dec_emb"][T], np.float32), (BL, D))),
        "egib": np.ascontiguousarray(egib).astype(BF16),
        "dgib": np.ascontiguousarray(dgib).astype(BF16),
        "ebhn": np.ascontiguousarray(enc_bhh[2 * H:][None, :]).astype(BF16),
        "dbhn": np.ascontiguousarray(dec_bhh[2 * H:][None, :]).astype(BF16),
        "combb": np.ascontiguousarray(combb).astype(BF16),
        "attnb": np.ascontiguousarray(f32("attn_b")[None, :]).astype(BF16),
        "lgb": np.ascontiguousarray(lgb).astype(BF16),
    }
    istk = np.zeros((128, BL), np.float32)
    istk[np.arange(128), np.arange(128) % BL] = 1.0
    common["istk"] = istk.astype(BF16)
    in_maps = []
    for c in range(NC):
        tok = tokens[c * BL:(c + 1) * BL].astype(np.int64)        # (64,25)
        xg = w2v[tok]                                             # (64,25,300)
        xT = np.zeros((E, MT * 128), np.float32)
        # column index = l*64 + b
        xT[:, :L * BL] = xg.transpose(2, 1, 0).reshape(E, L * BL)
        m = dict(common)
        m["xT"] = xT
        in_maps.append(m)

    nc = build_nc()
    trace = bool(int(os.environ.get("KERNEL_TRACE", "0")))
    res = bass_utils.run_bass_kernel_spmd(nc, in_maps, core_ids=list(range(NC)),
                                          trace=trace)
    if trace and res.exec_time_ns is not None:
        print(f"HW exec time: {res.exec_time_ns} ns", flush=True)
        print("trace:", res.instructions_and_trace[1] if res.instructions_and_trace else None,
              flush=True)
    out = np.concatenate([res.results[c]["out"] for c in range(NC)], axis=0)
    return out.astype(np.float32)


if __name__ == "__main__":
    pass



# revision 13
# speedup vs baseline: 2.0014x; 2.0014x over previous
"""Trainium2 Bass kernel for nn_Attention_72670846649042.

GRU encoder + greedy attention decoder, B=512,L=25,H=1024,D=256,T=128,E=300.
Sharding: data-parallel over batch, 64 rows/core on 8 cores, no collectives.

v2 design (vs v1 baseline at 2.80 ms):
 - No DMA transposes. State transposes run on the PE (128x128 transpose of a
   "folded" [128,128] slice yields two hT k-tiles at once). Keeps HAM warm.
 - Folded layout: batch-halves of the hidden dim live at PSUM/SBUF partitions
   0:64 / 64:128. M=64 matmuls are column-packed in pairs (tile_position via
   out.base_partition), halving PE passes; DVE gate math runs at 128 lanes.
 - Encoder input projection (x @ Wih + bias) inlined into the scan as three
   extra k-tiles per gate (bias folded as a ones-row of xT). No DRAM bounce.
 - comb_W (applied part) folded into enc_out once after the encoder (encC),
   so the attention einsum directly produces the comb output.
 - Activation-table discipline: encoder uses {sigmoid,tanh}, decoder uses
   {exp,tanh,relu} (sigmoid via tanh identity), log-softmax ln deferred to a
   single batched tail. Zero table reloads inside the loops.
"""
import os
import numpy as np
import ml_dtypes

B, L, V, E, H, D, T = 512, 25, 50000, 300, 1024, 256, 128
NC = 8
BL = B // NC          # 64 local batch
G3 = 3 * H            # 3072
KH = H // 128         # 8 hidden ktiles
MT = 13               # l-pair tiles for attention (25 -> 13 pairs, last padded)
MAXN1, MAXN2, BN_EPS = 10.0, 1.0, 1e-5
EK = (128, 128, 45)   # xT/encWih ktile rows (300 rows + 1 ones row)
BF16 = ml_dtypes.bfloat16

LINEARIZE = False


def build_nc():
    import concourse.bass as bass
    import concourse.tile as tile
    from concourse import bacc, mybir
    from contextlib import ExitStack

    dt = mybir.dt
    AF = mybir.ActivationFunctionType
    ALU = mybir.AluOpType
    AX = mybir.AxisListType

    nc = bacc.Bacc("TRN2", target_bir_lowering=False, debug=False)

    # ---- dram parameters ----
    xTb_d = nc.declare_dram_parameter("xTb", [301, L * BL], dt.bfloat16, isOutput=False)
    encWihT_d = nc.declare_dram_parameter("encWihT", [301, G3], dt.bfloat16, isOutput=False)
    encWhhT_d = nc.declare_dram_parameter("encWhhT", [H, G3], dt.bfloat16, isOutput=False)
    decWihT_d = nc.declare_dram_parameter("decWihT", [H, G3], dt.bfloat16, isOutput=False)
    decWhhT_d = nc.declare_dram_parameter("decWhhT", [H, G3], dt.bfloat16, isOutput=False)
    combWembT_d = nc.declare_dram_parameter("combWembT", [D, H], dt.bfloat16, isOutput=False)
    combWappT_d = nc.declare_dram_parameter("combWappT", [H, H], dt.bfloat16, isOutput=False)
    outWTs_d = nc.declare_dram_parameter("outWTs", [H, T], dt.bfloat16, isOutput=False)
    attnWT_d = nc.declare_dram_parameter("attnWT", [D + H, L], dt.bfloat16, isOutput=False)
    embf_d = nc.declare_dram_parameter("embf", [128, D], dt.bfloat16, isOutput=False)
    embT0_d = nc.declare_dram_parameter("embT0", [128, 2 * BL], dt.bfloat16, isOutput=False)
    ident_d = nc.declare_dram_parameter("ident", [128, 128], dt.bfloat16, isOutput=False)
    istk_d = nc.declare_dram_parameter("istk", [128, BL], dt.bfloat16, isOutput=False)
    ebhn_d = nc.declare_dram_parameter("ebhn", [1, H], dt.bfloat16, isOutput=False)
    dgib_d = nc.declare_dram_parameter("dgib", [1, G3], dt.bfloat16, isOutput=False)
    dbhn_d = nc.declare_dram_parameter("dbhn", [1, H], dt.bfloat16, isOutput=False)
    combb_d = nc.declare_dram_parameter("combb", [1, H], dt.bfloat16, isOutput=False)
    attnb_d = nc.declare_dram_parameter("attnb", [1, L], dt.bfloat16, isOutput=False)
    lgb_d = nc.declare_dram_parameter("lgb", [1, T], dt.bfloat16, isOutput=False)
    out_d = nc.declare_dram_parameter("out", [BL * L, T], dt.float32, isOutput=True)

    with tile.TileContext(nc, linearize=LINEARIZE) as tc, ExitStack() as ctx:
        shared = ctx.enter_context(tc.tile_pool(name="shared", bufs=1))
        decw = ctx.enter_context(tc.tile_pool(name="decw", bufs=1))

        ident = shared.tile([128, 128], dt.bfloat16, tag="ident")
        nc.sync.dma_start(ident[:], ident_d.ap())
        Istk = shared.tile([128, BL], dt.bfloat16, tag="Istk")
        nc.sync.dma_start(Istk[:], istk_d.ap())
        embf = shared.tile([128, D], dt.bfloat16, tag="embf")
        nc.sync.dma_start(embf[:], embf_d.ap())
        embT = shared.tile([128, 2, BL], dt.bfloat16, tag="embT")
        nc.sync.dma_start(embT[:], embT0_d.ap())
        attnWT = shared.tile([128, 10, L], dt.bfloat16, tag="attnWT")
        nc.sync.dma_start(attnWT[:], attnWT_d.ap().rearrange("(k p) n -> p k n", p=128))
        ones_sb = shared.tile([1, 128], dt.bfloat16, tag="ones_sb")
        nc.vector.memset(ones_sb[:], 1.0)

        ebhn_r = shared.tile([1, H], dt.bfloat16, tag="ebhn_r")
        nc.sync.dma_start(ebhn_r[:], ebhn_d.ap())
        dgib_r = shared.tile([1, G3], dt.bfloat16, tag="dgib_r")
        nc.sync.dma_start(dgib_r[:], dgib_d.ap())
        dbhn_r = shared.tile([1, H], dt.bfloat16, tag="dbhn_r")
        nc.sync.dma_start(dbhn_r[:], dbhn_d.ap())
        combb_r = shared.tile([1, H], dt.bfloat16, tag="combb_r")
        nc.sync.dma_start(combb_r[:], combb_d.ap())
        attnb_r = shared.tile([1, L], dt.bfloat16, tag="attnb_r")
        nc.sync.dma_start(attnb_r[:], attnb_d.ap())
        lgb_r = shared.tile([1, T], dt.bfloat16, tag="lgb_r")
        nc.sync.dma_start(lgb_r[:], lgb_d.ap())

        hA = shared.tile([128, 512], dt.float32, tag="hA")
        hB = shared.tile([128, 512], dt.float32, tag="hB")
        nc.vector.memset(hA[:], 0.0)
        h_tiles = [hA, hB]
        h2bf = shared.tile([128, 512], dt.bfloat16, tag="h2bf")

        se_all = shared.tile([BL, L], dt.float32, tag="se_all")
        mx_all = shared.tile([BL, L], dt.float32, tag="mx_all")

        # decoder weights: decWhhT prefetched during encoder; rest at encC.
        decWhhT = decw.tile([128, KH, G3], dt.bfloat16, tag="decWhhT")
        nc.sync.dma_start(decWhhT[:], decWhhT_d.ap().rearrange("(k p) n -> p k n", p=128))
        combWembT = decw.tile([128, 2, H], dt.bfloat16, tag="combWembT")
        nc.sync.dma_start(combWembT[:], combWembT_d.ap().rearrange("(k p) n -> p k n", p=128))
        outWTs = decw.tile([128, KH, T], dt.bfloat16, tag="outWTs")
        nc.sync.dma_start(outWTs[:], outWTs_d.ap().rearrange("(k p) n -> p k n", p=128))

        # encoder output history, transposed: [h-slice part, k, l, b], l=25 + pad
        enc_outT = decw.tile([128, KH, 26, BL], dt.bfloat16, tag="enc_outT")
        nc.vector.memset(enc_outT[:, :, 25, :], 0.0)

        def fold_transposes(src_bf, tp):
            # src_bf folded [128,512]; tp psum [128, 4, 2, BL]: one 128x128
            # transpose per 128-col block yields k-tiles (f, f+4).
            for f in range(4):
                nc.tensor.transpose(tp[:, f, :, :], src_bf[:, f * 128:(f + 1) * 128],
                                    ident[:])

        # =======================================================
        # Phase 1: encoder scan (gi inlined; 25 steps)
        # =======================================================
        with tc.tile_pool(name="encw", bufs=1) as encw, \
             tc.tile_pool(name="egps", bufs=2, space="PSUM") as egps, \
             tc.tile_pool(name="egp1", bufs=1, space="PSUM") as egp1, \
             tc.tile_pool(name="tpp", bufs=1, space="PSUM") as tpp, \
             tc.tile_pool(name="ework", bufs=2) as ework:
            xT = encw.tile([128, 3, L * BL], dt.bfloat16, tag="xT")
            nc.sync.dma_start(xT[:, 0, :], xTb_d.ap()[0:128, :])
            nc.sync.dma_start(xT[:, 1, :], xTb_d.ap()[128:256, :])
            nc.sync.dma_start(xT[0:45, 2, :], xTb_d.ap()[256:301, :])
            eWih = encw.tile([128, 3, G3], dt.bfloat16, tag="eWih")
            nc.sync.dma_start(eWih[:, 0, :], encWihT_d.ap()[0:128, :])
            nc.sync.dma_start(eWih[:, 1, :], encWihT_d.ap()[128:256, :])
            nc.sync.dma_start(eWih[0:45, 2, :], encWihT_d.ap()[256:301, :])
            eWhh = encw.tile([128, KH, G3], dt.bfloat16, tag="eWhh")
            nc.sync.dma_start(eWhh[:], encWhhT_d.ap().rearrange("(k p) n -> p k n", p=128))

            def emit_gi(t, ps_r, ps_z, ps_ngi, rz_stop):
                for g, bank, stp in ((0, ps_r, rz_stop), (1, ps_z, rz_stop),
                                     (2, ps_ngi, True)):
                    for hc in range(2):
                        o = bank[hc * 64:(hc + 1) * 64, :]
                        co = g * H + hc * 512
                        for kt in range(3):
                            nc.tensor.matmul(
                                o, xT[0:EK[kt], kt, t * BL:(t + 1) * BL],
                                eWih[0:EK[kt], kt, co:co + 512],
                                start=(kt == 0), stop=(stp and kt == 2))

            def alloc_banks():
                return (egps.tile([128, 512], dt.float32, name="ps_r", tag="r"),
                        egps.tile([128, 512], dt.float32, name="ps_z", tag="z"),
                        egps.tile([128, 512], dt.float32, name="ps_ngi", tag="ngi"))

            banks = {}
            banks[0] = alloc_banks()
            emit_gi(0, *banks[0], rz_stop=True)
            for t in range(L):
                ps_r, ps_z, ps_ngi = banks.pop(t)
                ps_ngh = egp1.tile([128, 512], dt.float32, tag="ngh")
                # gh matmuls (skip at t=0: h=0)
                if t > 0:
                    for g, bank in ((0, ps_r), (1, ps_z)):
                        for hc in range(2):
                            o = bank[hc * 64:(hc + 1) * 64, :]
                            co = g * H + hc * 512
                            for k in range(KH):
                                nc.tensor.matmul(
                                    o, enc_outT[:, k, t - 1, :],
                                    eWhh[:, k, co:co + 512],
                                    start=False, stop=(k == KH - 1))
                    for hc in range(2):
                        o = ps_ngh[hc * 64:(hc + 1) * 64, :]
                        co = 2 * H + hc * 512
                        for k in range(KH):
                            nc.tensor.matmul(o, enc_outT[:, k, t - 1, :],
                                             eWhh[:, k, co:co + 512],
                                             start=(k == 0), stop=False)
                for hc in range(2):
                    nc.tensor.matmul(ps_ngh[hc * 64:(hc + 1) * 64, :],
                                     ones_sb[0:1, 0:BL],
                                     ebhn_r[0:1, hc * 512:hc * 512 + 512],
                                     start=(t == 0), stop=True)
                # next step's gi (fills PE while this step's gate chain runs)
                if t + 1 < L:
                    banks[t + 1] = alloc_banks()
                    emit_gi(t + 1, *banks[t + 1], rz_stop=False)
                # ---- gates (folded [128,512]) ----
                hprev = h_tiles[t % 2]
                hnew = h_tiles[(t + 1) % 2]
                r_s = ework.tile([128, 512], dt.bfloat16, tag="r_s")
                nc.scalar.activation(r_s[:], ps_r[:], AF.Sigmoid)
                z_s = ework.tile([128, 512], dt.bfloat16, tag="z_s")
                nc.scalar.activation(z_s[:], ps_z[:], AF.Sigmoid)
                nt = ework.tile([128, 512], dt.float32, tag="nt")
                nc.vector.tensor_tensor(nt[:], ps_ngh[:], r_s[:], op=ALU.mult)
                nc.vector.tensor_tensor(nt[:], nt[:], ps_ngi[:], op=ALU.add)
                n_s = ework.tile([128, 512], dt.float32, tag="n_s")
                nc.scalar.activation(n_s[:], nt[:], AF.Tanh)
                t4 = ework.tile([128, 512], dt.float32, tag="t4")
                nc.vector.tensor_tensor(t4[:], hprev[:], n_s[:], op=ALU.subtract)
                nc.vector.tensor_tensor(t4[:], t4[:], z_s[:], op=ALU.mult)
                nc.vector.tensor_tensor(hnew[:], n_s[:], t4[:], op=ALU.add)
                nc.scalar.copy(h2bf[:], hnew[:])
                # ---- transpose h2 -> enc_outT[:, :, t, :] ----
                tp = tpp.tile([128, 4, 2, BL], dt.bfloat16, tag="tp")
                fold_transposes(h2bf, tp)
                nc.vector.tensor_copy(
                    enc_outT[:, :, t, :].rearrange("p (hc f) b -> p hc f b", hc=2),
                    tp[:].rearrange("p f hc b -> p hc f b"))

        # =======================================================
        # Phase 2: encC = enc_out @ combW_app   (+ load decoder weights)
        # =======================================================
        decw2 = ctx.enter_context(tc.tile_pool(name="decw2", bufs=1))
        decWihT = decw2.tile([128, KH, G3], dt.bfloat16, tag="decWihT")
        nc.sync.dma_start(decWihT[:], decWihT_d.ap().rearrange("(k p) n -> p k n", p=128))
        encC = decw2.tile([128, MT, H], dt.bfloat16, tag="encC")
        lg_all = decw2.tile([BL, L, T], dt.float32, tag="lg_all")
        hTt = decw2.tile([128, KH, BL], dt.bfloat16, tag="hTt")
        oTt = decw2.tile([128, KH, BL], dt.bfloat16, tag="oTt")
        with tc.tile_pool(name="ccw", bufs=1) as ccw, \
             tc.tile_pool(name="ccps", bufs=4, space="PSUM") as ccps:
            combWappT = ccw.tile([128, KH, H], dt.bfloat16, tag="combWappT")
            nc.sync.dma_start(combWappT[:],
                              combWappT_d.ap().rearrange("(k p) n -> p k n", p=128))
            for m in range(MT):
                for nch in range(2):
                    ps = ccps.tile([128, 512], dt.float32, tag="cc")
                    for k in range(KH):
                        nc.tensor.matmul(
                            ps[:], enc_outT[:, k, 2 * m:2 * m + 2, :],
                            combWappT[:, k, nch * 512:(nch + 1) * 512],
                            start=(k == 0), stop=(k == KH - 1))
                    nc.vector.tensor_copy(encC[:, m, nch * 512:(nch + 1) * 512], ps[:])
        nc.vector.tensor_copy(hTt[:], enc_outT[:, :, 24, :])

        # =======================================================
        # Phase 3: decoder (25 steps)
        # =======================================================
        with tc.tile_pool(name="dgps", bufs=1, space="PSUM") as dgps, \
             tc.tile_pool(name="dops", bufs=1, space="PSUM") as dops, \
             tc.tile_pool(name="tpp2", bufs=1, space="PSUM") as tpp2, \
             tc.tile_pool(name="mscp", bufs=1, space="PSUM") as mscp, \
             tc.tile_pool(name="lgps", bufs=1, space="PSUM") as lgps, \
             tc.tile_pool(name="dwork", bufs=2) as dwork:
            for t in range(L):
                # ---- attention scores -> misc[0:64, 128:153] ----
                misc = mscp.tile([128, 512], dt.float32, tag="misc")
                sc = misc[0:BL, 128:128 + L]
                for j in range(2):
                    nc.tensor.matmul(sc, embT[:, j, :], attnWT[:, j, :],
                                     start=(j == 0), stop=False)
                for k in range(KH):
                    nc.tensor.matmul(sc, hTt[:, k, :], attnWT[:, 2 + k, :],
                                     start=False, stop=False)
                nc.tensor.matmul(sc, ones_sb[0:1, 0:BL], attnb_r[:],
                                 start=False, stop=True)
                # ---- gh matmuls (r,z from hT; ngh + bias) ----
                ps_r = dgps.tile([128, 512], dt.float32, tag="r")
                ps_z = dgps.tile([128, 512], dt.float32, tag="z")
                ps_ngh = dgps.tile([128, 512], dt.float32, tag="ngh")
                ps_ngi = dgps.tile([128, 512], dt.float32, tag="ngi")
                for g, bank in ((0, ps_r), (1, ps_z)):
                    for hc in range(2):
                        o = bank[hc * 64:(hc + 1) * 64, :]
                        co = g * H + hc * 512
                        for k in range(KH):
                            nc.tensor.matmul(o, hTt[:, k, :], decWhhT[:, k, co:co + 512],
                                             start=(k == 0), stop=False)
                for hc in range(2):
                    o = ps_ngh[hc * 64:(hc + 1) * 64, :]
                    co = 2 * H + hc * 512
                    for k in range(KH):
                        nc.tensor.matmul(o, hTt[:, k, :], decWhhT[:, k, co:co + 512],
                                         start=(k == 0), stop=False)
                    nc.tensor.matmul(o, ones_sb[0:1, 0:BL],
                                     dbhn_r[0:1, hc * 512:hc * 512 + 512],
                                     start=False, stop=True)
                # ---- softmax over scores ----
                mx = dwork.tile([BL, 1], dt.float32, tag="mx")
                nc.vector.tensor_reduce(mx[:], sc, axis=AX.X, op=ALU.max)
                nmx = dwork.tile([BL, 1], dt.float32, tag="nmx")
                nc.vector.tensor_scalar(nmx[:], mx[:], -1.0, None, op0=ALU.mult)
                aw = dwork.tile([BL, L], dt.float32, tag="aw")
                sume = dwork.tile([BL, 1], dt.float32, tag="sume")
                nc.scalar.activation(aw[:], sc, AF.Exp, bias=nmx[:], accum_out=sume[:])
                rs = dwork.tile([BL, 1], dt.float32, tag="rs")
                nc.vector.reciprocal(rs[:], sume[:])
                awsh = dwork.tile([128, L], dt.float32, tag="awsh")
                nc.vector.memset(awsh[BL:128, L - 1:L], 0.0)
                nc.vector.tensor_copy(awsh[0:BL, :], aw[:])
                nc.vector.tensor_copy(awsh[BL:128, 0:L - 1], aw[:, 1:L])
                rs2 = dwork.tile([128, 1], dt.float32, tag="rs2")
                nc.vector.tensor_copy(rs2[0:BL, :], rs[:])
                nc.vector.tensor_copy(rs2[BL:128, :], rs[:])
                # ---- o = emb@combWemb + einsum(aw, encC) + combb ----
                ps_o = dops.tile([128, 512], dt.float32, tag="o")
                for j in range(2):
                    for hc in range(2):
                        nc.tensor.matmul(ps_o[hc * 64:(hc + 1) * 64, :],
                                         embT[:, j, :],
                                         combWembT[:, j, hc * 512:(hc + 1) * 512],
                                         start=(j == 0), stop=False)
                for hc in range(2):
                    nc.tensor.matmul(ps_o[hc * 64:(hc + 1) * 64, :],
                                     ones_sb[0:1, 0:BL],
                                     combb_r[0:1, hc * 512:hc * 512 + 512],
                                     start=False, stop=False)
                dgs = dwork.tile([128, MT * BL], dt.bfloat16, tag="dgs", bufs=1)
                for p in range(MT):
                    nc.vector.tensor_scalar(dgs[:, p * BL:(p + 1) * BL], Istk[:],
                                            awsh[:, 2 * p:2 * p + 1], rs2[:],
                                            op0=ALU.mult, op1=ALU.mult)
                for p in range(MT):
                    for hc in range(2):
                        nc.tensor.matmul(ps_o[hc * 64:(hc + 1) * 64, :],
                                         dgs[:, p * BL:(p + 1) * BL],
                                         encC[:, p, hc * 512:(hc + 1) * 512],
                                         start=False, stop=(p == MT - 1))
                obf = dwork.tile([128, 512], dt.bfloat16, tag="obf")
                nc.scalar.activation(obf[:], ps_o[:], AF.Relu, scale=S2_SCALE)
                # ---- oT ----
                tp = tpp2.tile([128, 4, 2, BL], dt.bfloat16, tag="tp")
                fold_transposes(obf, tp)
                nc.vector.tensor_copy(
                    oTt[:].rearrange("p (hc f) b -> p hc f b", hc=2),
                    tp[:].rearrange("p f hc b -> p hc f b"))
                # ---- gi matmuls from oT (+ biases) ----
                for g, bank in ((0, ps_r), (1, ps_z), (2, ps_ngi)):
                    for hc in range(2):
                        o = bank[hc * 64:(hc + 1) * 64, :]
                        co = g * H + hc * 512
                        for k in range(KH):
                            nc.tensor.matmul(o, oTt[:, k, :], decWihT[:, k, co:co + 512],
                                             start=(g == 2 and k == 0), stop=False)
                        nc.tensor.matmul(o, ones_sb[0:1, 0:BL],
                                         dgib_r[0:1, co:co + 512],
                                         start=False, stop=True)
                # ---- gates (sigma via tanh; folded [128,512]) ----
                hprev = h_tiles[(L + t) % 2]
                hnew = h_tiles[(L + t + 1) % 2]
                r_s = dwork.tile([128, 512], dt.bfloat16, tag="r_s")
                nc.scalar.activation(r_s[:], ps_r[:], AF.Tanh, scale=0.5)
                nc.vector.tensor_scalar(r_s[:], r_s[:], 0.5, 0.5, op0=ALU.mult,
                                        op1=ALU.add)
                z_s = dwork.tile([128, 512], dt.bfloat16, tag="z_s")
                nc.scalar.activation(z_s[:], ps_z[:], AF.Tanh, scale=0.5)
                nc.vector.tensor_scalar(z_s[:], z_s[:], 0.5, 0.5, op0=ALU.mult,
                                        op1=ALU.add)
                nt = dwork.tile([128, 512], dt.float32, tag="nt", bufs=1)
                nc.vector.tensor_tensor(nt[:], ps_ngh[:], r_s[:], op=ALU.mult)
                nc.vector.tensor_tensor(nt[:], nt[:], ps_ngi[:], op=ALU.add)
                n_s = dwork.tile([128, 512], dt.float32, tag="n_s", bufs=1)
                nc.scalar.activation(n_s[:], nt[:], AF.Tanh)
                t4 = dwork.tile([128, 512], dt.float32, tag="t4", bufs=1)
                nc.vector.tensor_tensor(t4[:], hprev[:], n_s[:], op=ALU.subtract)
                nc.vector.tensor_tensor(t4[:], t4[:], z_s[:], op=ALU.mult)
                nc.vector.tensor_tensor(hnew[:], n_s[:], t4[:], op=ALU.add)
                nc.scalar.copy(h2bf[:], hnew[:])
                # ---- hT refresh ----
                tp2 = tpp2.tile([128, 4, 2, BL], dt.bfloat16, tag="tp")
                fold_transposes(h2bf, tp2)
                nc.vector.tensor_copy(
                    hTt[:].rearrange("p (hc f) b -> p hc f b", hc=2),
                    tp2[:].rearrange("p f hc b -> p hc f b"))
                # ---- logits ----
                lg = lgps.tile([BL, T], dt.float32, tag="lg")
                for k in range(KH):
                    nc.tensor.matmul(lg[:], hTt[:, k, :], outWTs[:, k, :],
                                     start=(k == 0), stop=False)
                nc.tensor.matmul(lg[:], ones_sb[0:1, 0:BL], lgb_r[:],
                                 start=False, stop=True)
                nc.scalar.copy(lg_all[:, t, :], lg[:])
                nc.vector.tensor_reduce(mx_all[:, t:t + 1], lg[:], axis=AX.X,
                                        op=ALU.max)
                nmx2 = dwork.tile([BL, 1], dt.float32, tag="nmx2")
                nc.vector.tensor_scalar(nmx2[:], mx_all[:, t:t + 1], -1.0, None,
                                        op0=ALU.mult)
                ex = dwork.tile([BL, T], dt.float32, tag="ex")
                nc.scalar.activation(ex[:], lg[:], AF.Exp, bias=nmx2[:],
                                     accum_out=se_all[:, t:t + 1])
                # ---- argmax -> next embT ----
                if t < L - 1:
                    oh = dwork.tile([BL, T], dt.bfloat16, tag="oh")
                    nc.vector.tensor_scalar(oh[:], lg[:], mx_all[:, t:t + 1], None,
                                            op0=ALU.is_equal)
                    tp3 = tpp2.tile([128, 4, 2, BL], dt.bfloat16, tag="tp")
                    nc.tensor.transpose(tp3[:, 0, :, :], oh[:], ident[0:BL, :])
                    ohT = dwork.tile([128, BL], dt.bfloat16, tag="ohT")
                    nc.vector.tensor_copy(ohT[:], tp3[:, 0, 0, :])
                    for j in range(2):
                        for hc in range(2):
                            nc.tensor.matmul(
                                misc[hc * 64:(hc + 1) * 64, j * 64:(j + 1) * 64],
                                embf[:, j * 128 + hc * 64:j * 128 + hc * 64 + 64],
                                ohT[:], start=True, stop=True)
                    nc.vector.tensor_copy(embT[:].rearrange("p j b -> p (j b)"),
                                          misc[:, 0:128])

        # =======================================================
        # Phase 4: log-softmax tail
        # =======================================================
        with tc.tile_pool(name="tail", bufs=2) as tail:
            lse = tail.tile([BL, L], dt.float32, tag="lse", bufs=1)
            nc.scalar.activation(lse[:], se_all[:], AF.Ln)
            nc.vector.tensor_tensor(lse[:], lse[:], mx_all[:], op=ALU.add)
            for t in range(L):
                lout = tail.tile([BL, T], dt.float32, tag="lout")
                nc.vector.tensor_scalar(lout[:], lg_all[:, t, :], lse[:, t:t + 1],
                                        None, op0=ALU.subtract)
                nc.sync.dma_start(
                    out_d.ap().rearrange("(b l) c -> b l c", l=L)[:, t, :], lout[:])
    nc.finalize()
    return nc


S2_SCALE = 1.0  # patched at build time (bn2 scale); module-level for closure use


def kernel(**inputs):
    global S2_SCALE
    import concourse.bass_utils as bass_utils

    tokens = np.asarray(inputs["tokens"])
    w2v = np.asarray(inputs["w2v"], np.float32)
    bn1 = np.asarray(inputs["bn1"], np.float32)
    bn2 = np.asarray(inputs["bn2"], np.float32)
    s1 = float(bn1[0] / np.sqrt(bn1[3] + BN_EPS))
    t1 = float(bn1[1] - bn1[2] * s1)
    s2 = float(bn2[0] / np.sqrt(bn2[3] + BN_EPS))
    t2 = float(bn2[1] - bn2[2] * s2)
    S2_SCALE = s2

    f32 = lambda k: np.asarray(inputs[k], np.float32)
    bft = lambda a: np.ascontiguousarray(np.asarray(a, np.float32).T).astype(BF16)
    enc_bih, enc_bhh = f32("enc_bih"), f32("enc_bhh")
    dec_bih, dec_bhh = f32("dec_bih"), f32("dec_bhh")
    egib = np.concatenate([enc_bih[:H] + enc_bhh[:H], enc_bih[H:2 * H] + enc_bhh[H:2 * H],
                           enc_bih[2 * H:]])
    dgib = np.concatenate([dec_bih[:H] + dec_bhh[:H], dec_bih[H:2 * H] + dec_bhh[H:2 * H],
                           dec_bih[2 * H:]])[None, :]
    out_W = f32("out_W")
    outWTs = np.ascontiguousarray((s1 * out_W).T).astype(BF16)
    lgb = (f32("out_b") + t1 * out_W.sum(axis=1))[None, :]
    combb = (f32("comb_b") + t2 / s2)[None, :]
    comb_W = f32("comb_W")

    # encoder Wih with bias folded as last row
    encWihT = np.zeros((301, G3), np.float32)
    encWihT[:300] = f32("enc_Wih").T
    encWihT[300] = egib

    # dec_emb rows 0..127 renormed (host)
    em = f32("dec_emb")[:128]
    emn = np.linalg.norm(em, axis=1, keepdims=True)
    embf = em * np.minimum(1.0, MAXN2 / (emn + 1e-7))
    # SOS embedding renormed, transposed, broadcast
    sos = f32("dec_emb")[T]
    sos = sos * min(1.0, MAXN2 / (np.linalg.norm(sos) + 1e-7))
    embT0 = np.broadcast_to(sos.reshape(2, 128).T[:, :, None], (128, 2, BL))

    ident = np.eye(128, dtype=np.float32)
    istk = np.zeros((128, BL), np.float32)
    istk[np.arange(128), np.arange(128) % BL] = 1.0

    common = {
        "encWihT": encWihT.astype(BF16), "encWhhT": bft(inputs["enc_Whh"]),
        "decWihT": bft(inputs["dec_Wih"]), "decWhhT": bft(inputs["dec_Whh"]),
        "combWembT": np.ascontiguousarray(comb_W[:, :D].T).astype(BF16),
        "combWappT": np.ascontiguousarray(comb_W[:, D:].T).astype(BF16),
        "outWTs": outWTs, "attnWT": bft(inputs["attn_W"]),
        "embf": np.ascontiguousarray(embf).astype(BF16),
        "embT0": np.ascontiguousarray(embT0.reshape(128, 2 * BL)).astype(BF16),
        "ident": ident.astype(BF16), "istk": istk.astype(BF16),
        "ebhn": np.ascontiguousarray(enc_bhh[2 * H:][None, :]).astype(BF16),
        "dgib": np.ascontiguousarray(dgib).astype(BF16),
        "dbhn": np.ascontiguousarray(dec_bhh[2 * H:][None, :]).astype(BF16),
        "combb": np.ascontiguousarray(combb).astype(BF16),
        "attnb": np.ascontiguousarray(f32("attn_b")[None, :]).astype(BF16),
        "lgb": np.ascontiguousarray(lgb).astype(BF16),
    }
    in_maps = []
    for c in range(NC):
        tok = tokens[c * BL:(c + 1) * BL].astype(np.int64)        # (64,25)
        xg = w2v[tok]                                             # (64,25,300)
        nrm = np.linalg.norm(xg, axis=-1, keepdims=True)
        xg = xg * np.minimum(1.0, MAXN1 / (nrm + 1e-7))
        xTb = np.zeros((301, L * BL), np.float32)
        xTb[:300] = xg.transpose(2, 1, 0).reshape(E, L * BL)      # col = l*64+b
        xTb[300] = 1.0
        m = dict(common)
        m["xTb"] = xTb.astype(BF16)
        in_maps.append(m)

    nc = build_nc()
    trace = bool(int(os.environ.get("KERNEL_TRACE", "0")))
    res = bass_utils.run_bass_kernel_spmd(nc, in_maps, core_ids=list(range(NC)),
                                          trace=trace)
    if trace and res.exec_time_ns is not None:
        print(f"HW exec time: {res.exec_time_ns} ns", flush=True)
        print("trace:", res.instructions_and_trace[1] if res.instructions_and_trace else None,
              flush=True)
    out = np.concatenate([res.results[c]["out"] for c in range(NC)], axis=0)
    return out.astype(np.float32)


if __name__ == "__main__":
    pass


# revision 14
# speedup vs baseline: 2.2386x; 1.1185x over previous
"""Trainium2 Bass kernel for nn_Attention_72670846649042.

GRU encoder + greedy attention decoder, B=512,L=25,H=1024,D=256,T=128,E=300.
Sharding: data-parallel over batch, 64 rows/core on 8 cores, no collectives.

v3 design (v1 baseline 2.80 ms, v2 1.40 ms):
 - No DMA transposes: PE 128x128 transposes of folded [128,128] slices of the
   fp32 state produce two hT k-tiles per instruction. Keeps HAM warm.
 - Folded layout: hidden-halves at partitions 0:64 / 64:128; M=64 matmuls are
   column-packed in pairs with the pair ADJACENT in the PE queue (concurrent
   col groups), halving PE passes; DVE gate math runs at 128 lanes.
 - Encoder input projection inlined into the scan (bias as a ones-row of xT).
 - comb_W (applied part) folded into enc_out once (encC): the attention
   einsum directly produces the comb output.
 - Gate chains chunked into 256-col halves: transposes / state copies /
   next-step matmuls (ktile order [0,4,1,5,2,6,3,7]) start after half 1.
 - Activation tables: encoder {sigmoid,tanh}, decoder {exp,tanh,relu}
   (sigmoid via tanh identity), ln deferred to one batched tail.
"""
import os
import numpy as np
import ml_dtypes

B, L, V, E, H, D, T = 512, 25, 50000, 300, 1024, 256, 128
NC = 8
BL = B // NC          # 64 local batch
G3 = 3 * H            # 3072
KH = H // 128         # 8 hidden ktiles
MT = 13               # l-pair tiles for attention (25 -> 13 pairs, last padded)
MAXN1, MAXN2, BN_EPS = 10.0, 1.0, 1e-5
EK = (128, 128, 45)   # xT/encWih ktile rows (300 rows + 1 ones row)
KORD = (0, 4, 1, 5, 2, 6, 3, 7)   # ktile order gated by chunk-half readiness
BF16 = ml_dtypes.bfloat16

LINEARIZE = False


def build_nc():
    import concourse.bass as bass
    import concourse.tile as tile
    from concourse import bacc, mybir
    from contextlib import ExitStack

    dt = mybir.dt
    AF = mybir.ActivationFunctionType
    ALU = mybir.AluOpType
    AX = mybir.AxisListType

    nc = bacc.Bacc("TRN2", target_bir_lowering=False, debug=False)

    # ---- dram parameters ----
    xTb_d = nc.declare_dram_parameter("xTb", [301, L * BL], dt.bfloat16, isOutput=False)
    encWihT_d = nc.declare_dram_parameter("encWihT", [301, G3], dt.bfloat16, isOutput=False)
    encWhhT_d = nc.declare_dram_parameter("encWhhT", [H, G3], dt.bfloat16, isOutput=False)
    decWihT_d = nc.declare_dram_parameter("decWihT", [H, G3], dt.bfloat16, isOutput=False)
    decWhhT_d = nc.declare_dram_parameter("decWhhT", [H, G3], dt.bfloat16, isOutput=False)
    combWembT_d = nc.declare_dram_parameter("combWembT", [D, H], dt.bfloat16, isOutput=False)
    combWappT_d = nc.declare_dram_parameter("combWappT", [H, H], dt.bfloat16, isOutput=False)
    outWTs_d = nc.declare_dram_parameter("outWTs", [H, T], dt.bfloat16, isOutput=False)
    attnWT_d = nc.declare_dram_parameter("attnWT", [D + H, L], dt.bfloat16, isOutput=False)
    embf_d = nc.declare_dram_parameter("embf", [128, D], dt.bfloat16, isOutput=False)
    embT0_d = nc.declare_dram_parameter("embT0", [128, 2 * BL], dt.bfloat16, isOutput=False)
    ident_d = nc.declare_dram_parameter("ident", [128, 128], dt.float32, isOutput=False)
    istkb_d = nc.declare_dram_parameter("istkb", [128, MT * BL], dt.bfloat16, isOutput=False)
    ebhn_d = nc.declare_dram_parameter("ebhn", [1, H], dt.bfloat16, isOutput=False)
    dgib_d = nc.declare_dram_parameter("dgib", [1, G3], dt.bfloat16, isOutput=False)
    dbhn_d = nc.declare_dram_parameter("dbhn", [1, H], dt.bfloat16, isOutput=False)
    combb_d = nc.declare_dram_parameter("combb", [1, H], dt.bfloat16, isOutput=False)
    attnb_d = nc.declare_dram_parameter("attnb", [1, L], dt.bfloat16, isOutput=False)
    lgb_d = nc.declare_dram_parameter("lgb", [1, T], dt.bfloat16, isOutput=False)
    out_d = nc.declare_dram_parameter("out", [BL * L, T], dt.float32, isOutput=True)

    with tile.TileContext(nc, linearize=LINEARIZE) as tc, ExitStack() as ctx:
        shared = ctx.enter_context(tc.tile_pool(name="shared", bufs=1))
        decw = ctx.enter_context(tc.tile_pool(name="decw", bufs=1))

        ident = shared.tile([128, 128], dt.float32, tag="ident")
        nc.sync.dma_start(ident[:], ident_d.ap())
        IstkB = shared.tile([128, MT, BL], dt.bfloat16, tag="IstkB")
        nc.sync.dma_start(IstkB[:], istkb_d.ap())
        embf = shared.tile([128, D], dt.bfloat16, tag="embf")
        nc.sync.dma_start(embf[:], embf_d.ap())
        embT = shared.tile([128, 2, BL], dt.bfloat16, tag="embT")
        nc.sync.dma_start(embT[:], embT0_d.ap())
        attnWT = shared.tile([128, 10, L], dt.bfloat16, tag="attnWT")
        nc.sync.dma_start(attnWT[:], attnWT_d.ap().rearrange("(k p) n -> p k n", p=128))
        ones_sb = shared.tile([1, 128], dt.bfloat16, tag="ones_sb")
        nc.vector.memset(ones_sb[:], 1.0)

        ebhn_r = shared.tile([1, H], dt.bfloat16, tag="ebhn_r")
        nc.sync.dma_start(ebhn_r[:], ebhn_d.ap())
        dgib_r = shared.tile([1, G3], dt.bfloat16, tag="dgib_r")
        nc.sync.dma_start(dgib_r[:], dgib_d.ap())
        dbhn_r = shared.tile([1, H], dt.bfloat16, tag="dbhn_r")
        nc.sync.dma_start(dbhn_r[:], dbhn_d.ap())
        combb_r = shared.tile([1, H], dt.bfloat16, tag="combb_r")
        nc.sync.dma_start(combb_r[:], combb_d.ap())
        attnb_r = shared.tile([1, L], dt.bfloat16, tag="attnb_r")
        nc.sync.dma_start(attnb_r[:], attnb_d.ap())
        lgb_r = shared.tile([1, T], dt.bfloat16, tag="lgb_r")
        nc.sync.dma_start(lgb_r[:], lgb_d.ap())

        hA = shared.tile([128, 512], dt.float32, tag="hA")
        hB = shared.tile([128, 512], dt.float32, tag="hB")
        nc.vector.memset(hA[:], 0.0)
        h_tiles = [hA, hB]

        se_all = shared.tile([BL, L], dt.float32, tag="se_all")
        mx_all = shared.tile([BL, L], dt.float32, tag="mx_all")

        # decoder weights: decWhhT prefetched during encoder; rest at encC.
        decWhhT = decw.tile([128, KH, G3], dt.bfloat16, tag="decWhhT")
        nc.sync.dma_start(decWhhT[:], decWhhT_d.ap().rearrange("(k p) n -> p k n", p=128))
        combWembT = decw.tile([128, 2, H], dt.bfloat16, tag="combWembT")
        nc.sync.dma_start(combWembT[:], combWembT_d.ap().rearrange("(k p) n -> p k n", p=128))
        outWTs = decw.tile([128, KH, T], dt.bfloat16, tag="outWTs")
        nc.sync.dma_start(outWTs[:], outWTs_d.ap().rearrange("(k p) n -> p k n", p=128))

        # encoder output history, transposed: [h-slice part, k, l, b], l=25 + pad
        enc_outT = decw.tile([128, KH, 26, BL], dt.bfloat16, tag="enc_outT")
        nc.vector.memset(enc_outT[:, :, 25, :], 0.0)

        def transp_half(hsrc, tp, c):
            # hsrc folded fp32 [128,512]; chunk c covers k-tiles {2c, 2c+1}
            # (partitions 0:64) and {2c+4, 2c+5} (partitions 64:128).
            for f in (2 * c, 2 * c + 1):
                nc.tensor.transpose(tp[:, f, :, :], hsrc[:, f * 128:(f + 1) * 128],
                                    ident[:])

        def copy_half(dst_kslices, tp, c):
            # dst view [128, hc, f(2), b] for f in {2c, 2c+1}
            nc.vector.tensor_copy(dst_kslices,
                                  tp[:, 2 * c:2 * c + 2, :, :].rearrange(
                                      "p f hc b -> p hc f b"))

        # =======================================================
        # Phase 1: encoder scan (gi inlined; 25 steps)
        # =======================================================
        with tc.tile_pool(name="encw", bufs=1) as encw, \
             tc.tile_pool(name="egps", bufs=2, space="PSUM") as egps, \
             tc.tile_pool(name="egp1", bufs=1, space="PSUM") as egp1, \
             tc.tile_pool(name="tpp", bufs=1, space="PSUM") as tpp, \
             tc.tile_pool(name="ework", bufs=2) as ework:
            xT = encw.tile([128, 3, L * BL], dt.bfloat16, tag="xT")
            nc.sync.dma_start(xT[:, 0, :], xTb_d.ap()[0:128, :])
            nc.sync.dma_start(xT[:, 1, :], xTb_d.ap()[128:256, :])
            nc.sync.dma_start(xT[0:45, 2, :], xTb_d.ap()[256:301, :])
            eWih = encw.tile([128, 3, G3], dt.bfloat16, tag="eWih")
            nc.sync.dma_start(eWih[:, 0, :], encWihT_d.ap()[0:128, :])
            nc.sync.dma_start(eWih[:, 1, :], encWihT_d.ap()[128:256, :])
            nc.sync.dma_start(eWih[0:45, 2, :], encWihT_d.ap()[256:301, :])
            eWhh = encw.tile([128, KH, G3], dt.bfloat16, tag="eWhh")
            nc.sync.dma_start(eWhh[:], encWhhT_d.ap().rearrange("(k p) n -> p k n", p=128))

            def emit_gi(t, ps_r, ps_z, ps_ngi, rz_stop):
                for g, bank, stp in ((0, ps_r, rz_stop), (2, ps_ngi, True),
                                     (1, ps_z, rz_stop)):
                    for kt in range(3):
                        for hc in range(2):
                            co = g * H + hc * 512
                            nc.tensor.matmul(
                                bank[hc * 64:(hc + 1) * 64, :],
                                xT[0:EK[kt], kt, t * BL:(t + 1) * BL],
                                eWih[0:EK[kt], kt, co:co + 512],
                                start=(kt == 0), stop=(stp and kt == 2))

            def alloc_banks():
                return (egps.tile([128, 512], dt.float32, name="ps_r", tag="r"),
                        egps.tile([128, 512], dt.float32, name="ps_z", tag="z"),
                        egps.tile([128, 512], dt.float32, name="ps_ngi", tag="ngi"))

            banks = {}
            banks[0] = alloc_banks()
            emit_gi(0, *banks[0], rz_stop=True)
            for t in range(L):
                ps_r, ps_z, ps_ngi = banks.pop(t)
                ps_ngh = egp1.tile([128, 512], dt.float32, tag="ngh")
                # gh matmuls (skip at t=0: h=0); bank order r, ngh, z so the
                # r/ngh-dependent gate chain starts earliest.
                if t > 0:
                    for g, bank, st in ((0, ps_r, False), (2, ps_ngh, True),
                                        (1, ps_z, False)):
                        for ki, k in enumerate(KORD):
                            for hc in range(2):
                                co = g * H + hc * 512
                                nc.tensor.matmul(
                                    bank[hc * 64:(hc + 1) * 64, :],
                                    enc_outT[:, k, t - 1, :],
                                    eWhh[:, k, co:co + 512],
                                    start=(st and ki == 0),
                                    stop=(not st and ki == KH - 1))
                for hc in range(2):
                    nc.tensor.matmul(ps_ngh[hc * 64:(hc + 1) * 64, :],
                                     ones_sb[0:1, 0:BL],
                                     ebhn_r[0:1, hc * 512:hc * 512 + 512],
                                     start=(t == 0), stop=True)
                # next step's gi (fills PE while this step's gate chain runs)
                if t + 1 < L:
                    banks[t + 1] = alloc_banks()
                    emit_gi(t + 1, *banks[t + 1], rz_stop=False)
                # ---- gates, chunked in 256-col halves ----
                hprev = h_tiles[t % 2]
                hnew = h_tiles[(t + 1) % 2]
                r_s = ework.tile([128, 512], dt.bfloat16, tag="r_s")
                z_s = ework.tile([128, 512], dt.bfloat16, tag="z_s")
                nt = ework.tile([128, 512], dt.float32, tag="nt", bufs=1)
                n_s = ework.tile([128, 512], dt.float32, tag="n_s", bufs=1)
                t4 = ework.tile([128, 512], dt.float32, tag="t4", bufs=1)
                tp = tpp.tile([128, 4, 2, BL], dt.float32, tag="tp")
                eo_view = enc_outT[:, :, t, :].rearrange("p (hc f) b -> p hc f b", hc=2)
                for c in range(2):
                    sl = slice(c * 256, (c + 1) * 256)
                    nc.scalar.activation(r_s[:, sl], ps_r[:, sl], AF.Sigmoid)
                    nc.scalar.activation(z_s[:, sl], ps_z[:, sl], AF.Sigmoid)
                    nc.vector.tensor_tensor(nt[:, sl], ps_ngh[:, sl], r_s[:, sl],
                                            op=ALU.mult)
                    nc.vector.tensor_tensor(nt[:, sl], nt[:, sl], ps_ngi[:, sl],
                                            op=ALU.add)
                    nc.scalar.activation(n_s[:, sl], nt[:, sl], AF.Tanh)
                    nc.vector.tensor_tensor(t4[:, sl], hprev[:, sl], n_s[:, sl],
                                            op=ALU.subtract)
                    nc.vector.tensor_tensor(t4[:, sl], t4[:, sl], z_s[:, sl],
                                            op=ALU.mult)
                    nc.vector.tensor_tensor(hnew[:, sl], n_s[:, sl], t4[:, sl],
                                            op=ALU.add)
                    transp_half(hnew, tp, c)
                    copy_half(eo_view[:, :, 2 * c:2 * c + 2, :], tp, c)

        # =======================================================
        # Phase 2: encC = enc_out @ combW_app   (+ load decoder weights)
        # =======================================================
        decw2 = ctx.enter_context(tc.tile_pool(name="decw2", bufs=1))
        decWihT = decw2.tile([128, KH, G3], dt.bfloat16, tag="decWihT")
        nc.sync.dma_start(decWihT[:], decWihT_d.ap().rearrange("(k p) n -> p k n", p=128))
        encC = decw2.tile([128, MT, H], dt.bfloat16, tag="encC")
        lg_all = decw2.tile([BL, L, T], dt.float32, tag="lg_all")
        hTt = decw2.tile([128, KH, BL], dt.bfloat16, tag="hTt")
        oTt = decw2.tile([128, KH, BL], dt.bfloat16, tag="oTt")
        with tc.tile_pool(name="ccw", bufs=1) as ccw, \
             tc.tile_pool(name="ccps", bufs=4, space="PSUM") as ccps:
            combWappT = ccw.tile([128, KH, H], dt.bfloat16, tag="combWappT")
            nc.sync.dma_start(combWappT[:],
                              combWappT_d.ap().rearrange("(k p) n -> p k n", p=128))
            for m in range(MT):
                for nch in range(2):
                    ps = ccps.tile([128, 512], dt.float32, tag="cc")
                    for k in range(KH):
                        nc.tensor.matmul(
                            ps[:], enc_outT[:, k, 2 * m:2 * m + 2, :],
                            combWappT[:, k, nch * 512:(nch + 1) * 512],
                            start=(k == 0), stop=(k == KH - 1))
                    nc.vector.tensor_copy(encC[:, m, nch * 512:(nch + 1) * 512], ps[:])
        nc.vector.tensor_copy(hTt[:], enc_outT[:, :, 24, :])

        # =======================================================
        # Phase 3: decoder (25 steps)
        # =======================================================
        with tc.tile_pool(name="dgps", bufs=1, space="PSUM") as dgps, \
             tc.tile_pool(name="dops", bufs=1, space="PSUM") as dops, \
             tc.tile_pool(name="tpp2", bufs=1, space="PSUM") as tpp2, \
             tc.tile_pool(name="mscp", bufs=1, space="PSUM") as mscp, \
             tc.tile_pool(name="lgps", bufs=1, space="PSUM") as lgps, \
             tc.tile_pool(name="dwork", bufs=2) as dwork:
            for t in range(L):
                # ---- attention scores -> misc[0:64, 128:153] ----
                misc = mscp.tile([128, 512], dt.float32, tag="misc")
                sc = misc[0:BL, 128:128 + L]
                for j in range(2):
                    nc.tensor.matmul(sc, embT[:, j, :], attnWT[:, j, :],
                                     start=(j == 0), stop=False)
                for k in KORD:
                    nc.tensor.matmul(sc, hTt[:, k, :], attnWT[:, 2 + k, :],
                                     start=False, stop=False)
                nc.tensor.matmul(sc, ones_sb[0:1, 0:BL], attnb_r[:],
                                 start=False, stop=True)
                # ---- gh matmuls (paired col groups) ----
                ps_r = dgps.tile([128, 512], dt.float32, tag="r")
                ps_z = dgps.tile([128, 512], dt.float32, tag="z")
                ps_ngh = dgps.tile([128, 512], dt.float32, tag="ngh")
                ps_ngi = dgps.tile([128, 512], dt.float32, tag="ngi")
                for g, bank in ((0, ps_r), (1, ps_z), (2, ps_ngh)):
                    for ki, k in enumerate(KORD):
                        for hc in range(2):
                            co = g * H + hc * 512
                            nc.tensor.matmul(bank[hc * 64:(hc + 1) * 64, :],
                                             hTt[:, k, :], decWhhT[:, k, co:co + 512],
                                             start=(ki == 0), stop=False)
                for hc in range(2):
                    nc.tensor.matmul(ps_ngh[hc * 64:(hc + 1) * 64, :],
                                     ones_sb[0:1, 0:BL],
                                     dbhn_r[0:1, hc * 512:hc * 512 + 512],
                                     start=False, stop=True)
                # ---- softmax over scores ----
                mx = dwork.tile([BL, 1], dt.float32, tag="mx")
                nc.vector.tensor_reduce(mx[:], sc, axis=AX.X, op=ALU.max)
                nmx = dwork.tile([BL, 1], dt.float32, tag="nmx")
                nc.vector.tensor_scalar(nmx[:], mx[:], -1.0, None, op0=ALU.mult)
                aw = dwork.tile([BL, L], dt.float32, tag="aw")
                sume = dwork.tile([BL, 1], dt.float32, tag="sume")
                nc.scalar.activation(aw[:], sc, AF.Exp, bias=nmx[:], accum_out=sume[:])
                rs = dwork.tile([BL, 1], dt.float32, tag="rs")
                nc.vector.reciprocal(rs[:], sume[:])
                rs2 = dwork.tile([128, 1], dt.float32, tag="rs2")
                nc.vector.tensor_copy(rs2[0:BL, :], rs[:])
                nc.vector.tensor_copy(rs2[BL:128, :], rs[:])
                # awn[q, p] = aw[b, 2p + (q>=64)] / sum  (l=25 slot zero)
                awn = dwork.tile([128, MT], dt.float32, tag="awn")
                nc.vector.tensor_copy(awn[0:BL, :], aw[:, 0:25:2])
                nc.vector.tensor_copy(awn[BL:128, 0:12], aw[:, 1:25:2])
                nc.vector.memset(awn[BL:128, 12:13], 0.0)
                nc.vector.tensor_scalar(awn[:], awn[:], rs2[:], None, op0=ALU.mult)
                dgs = dwork.tile([128, MT, BL], dt.bfloat16, tag="dgs", bufs=1)
                nc.vector.tensor_tensor(dgs[:], IstkB[:],
                                        awn[:].broadcast_to((128, MT, BL)),
                                        op=ALU.mult)
                # ---- o = emb@combWemb + einsum(aw, encC) + combb ----
                ps_o = dops.tile([128, 512], dt.float32, tag="o")
                for j in range(2):
                    for hc in range(2):
                        nc.tensor.matmul(ps_o[hc * 64:(hc + 1) * 64, :],
                                         embT[:, j, :],
                                         combWembT[:, j, hc * 512:(hc + 1) * 512],
                                         start=(j == 0), stop=False)
                for hc in range(2):
                    nc.tensor.matmul(ps_o[hc * 64:(hc + 1) * 64, :],
                                     ones_sb[0:1, 0:BL],
                                     combb_r[0:1, hc * 512:hc * 512 + 512],
                                     start=False, stop=False)
                for p in range(MT):
                    for hc in range(2):
                        nc.tensor.matmul(ps_o[hc * 64:(hc + 1) * 64, :],
                                         dgs[:, p, :],
                                         encC[:, p, hc * 512:(hc + 1) * 512],
                                         start=False, stop=(p == MT - 1))
                obf = dwork.tile([128, 512], dt.float32, tag="obf")
                nc.scalar.activation(obf[:], ps_o[:], AF.Relu, scale=S2_SCALE)
                # ---- oT ----
                tp = tpp2.tile([128, 4, 2, BL], dt.float32, tag="tp")
                for c in range(2):
                    transp_half(obf, tp, c)
                nc.vector.tensor_copy(
                    oTt[:].rearrange("p (hc f) b -> p hc f b", hc=2),
                    tp[:].rearrange("p f hc b -> p hc f b"))
                # ---- gi matmuls from oT; bank order r, ngi, z ----
                for g, bank in ((0, ps_r), (2, ps_ngi), (1, ps_z)):
                    for ki, k in enumerate(KORD):
                        for hc in range(2):
                            co = g * H + hc * 512
                            nc.tensor.matmul(bank[hc * 64:(hc + 1) * 64, :],
                                             oTt[:, k, :], decWihT[:, k, co:co + 512],
                                             start=(g == 2 and ki == 0), stop=False)
                    for hc in range(2):
                        co = g * H + hc * 512
                        nc.tensor.matmul(bank[hc * 64:(hc + 1) * 64, :],
                                         ones_sb[0:1, 0:BL],
                                         dgib_r[0:1, co:co + 512],
                                         start=False, stop=True)
                # ---- gates (sigma via tanh), chunked halves ----
                hprev = h_tiles[(L + t) % 2]
                hnew = h_tiles[(L + t + 1) % 2]
                r_s = dwork.tile([128, 512], dt.bfloat16, tag="r_s")
                z_s = dwork.tile([128, 512], dt.bfloat16, tag="z_s")
                nt = dwork.tile([128, 512], dt.float32, tag="nt", bufs=1)
                n_s = dwork.tile([128, 512], dt.float32, tag="n_s", bufs=1)
                t4 = dwork.tile([128, 512], dt.float32, tag="t4", bufs=1)
                tp2 = tpp2.tile([128, 4, 2, BL], dt.float32, tag="tp")
                lg = lgps.tile([BL, T], dt.float32, tag="lg")
                hT_view = hTt[:].rearrange("p (hc f) b -> p hc f b", hc=2)
                for c in range(2):
                    sl = slice(c * 256, (c + 1) * 256)
                    nc.scalar.activation(r_s[:, sl], ps_r[:, sl], AF.Tanh, scale=0.5)
                    nc.vector.tensor_scalar(r_s[:, sl], r_s[:, sl], 0.5, 0.5,
                                            op0=ALU.mult, op1=ALU.add)
                    nc.scalar.activation(z_s[:, sl], ps_z[:, sl], AF.Tanh, scale=0.5)
                    nc.vector.tensor_scalar(z_s[:, sl], z_s[:, sl], 0.5, 0.5,
                                            op0=ALU.mult, op1=ALU.add)
                    nc.vector.tensor_tensor(nt[:, sl], ps_ngh[:, sl], r_s[:, sl],
                                            op=ALU.mult)
                    nc.vector.tensor_tensor(nt[:, sl], nt[:, sl], ps_ngi[:, sl],
                                            op=ALU.add)
                    nc.scalar.activation(n_s[:, sl], nt[:, sl], AF.Tanh)
                    nc.vector.tensor_tensor(t4[:, sl], hprev[:, sl], n_s[:, sl],
                                            op=ALU.subtract)
                    nc.vector.tensor_tensor(t4[:, sl], t4[:, sl], z_s[:, sl],
                                            op=ALU.mult)
                    nc.vector.tensor_tensor(hnew[:, sl], n_s[:, sl], t4[:, sl],
                                            op=ALU.add)
                    transp_half(hnew, tp2, c)
                    copy_half(hT_view[:, :, 2 * c:2 * c + 2, :], tp2, c)
                    # logits for the k-tiles this half provides
                    for ki, k in enumerate(KORD[4 * c:4 * c + 4]):
                        nc.tensor.matmul(lg[:], hTt[:, k, :], outWTs[:, k, :],
                                         start=(c == 0 and ki == 0), stop=False)
                nc.tensor.matmul(lg[:], ones_sb[0:1, 0:BL], lgb_r[:],
                                 start=False, stop=True)
                nc.scalar.copy(lg_all[:, t, :], lg[:])
                nc.vector.tensor_reduce(mx_all[:, t:t + 1], lg[:], axis=AX.X,
                                        op=ALU.max)
                nmx2 = dwork.tile([BL, 1], dt.float32, tag="nmx2")
                nc.vector.tensor_scalar(nmx2[:], mx_all[:, t:t + 1], -1.0, None,
                                        op0=ALU.mult)
                ex = dwork.tile([BL, T], dt.float32, tag="ex")
                nc.scalar.activation(ex[:], lg[:], AF.Exp, bias=nmx2[:],
                                     accum_out=se_all[:, t:t + 1])
                # ---- argmax -> next embT ----
                if t < L - 1:
                    oh = dwork.tile([BL, T], dt.float32, tag="oh")
                    nc.vector.tensor_scalar(oh[:], lg[:], mx_all[:, t:t + 1], None,
                                            op0=ALU.is_equal)
                    tp3 = tpp2.tile([128, 4, 2, BL], dt.float32, tag="tp")
                    nc.tensor.transpose(tp3[:, 0, :, :], oh[:], ident[0:BL, :])
                    ohT = dwork.tile([128, BL], dt.bfloat16, tag="ohT")
                    nc.vector.tensor_copy(ohT[:], tp3[:, 0, 0, :])
                    for j in range(2):
                        for hc in range(2):
                            nc.tensor.matmul(
                                misc[hc * 64:(hc + 1) * 64, j * 64:(j + 1) * 64],
                                embf[:, j * 128 + hc * 64:j * 128 + hc * 64 + 64],
                                ohT[:], start=True, stop=True)
                    nc.vector.tensor_copy(embT[:].rearrange("p j b -> p (j b)"),
                                          misc[:, 0:128])

        # =======================================================
        # Phase 4: log-softmax tail
        # =======================================================
        with tc.tile_pool(name="tail", bufs=2) as tail:
            lse = tail.tile([BL, L], dt.float32, tag="lse", bufs=1)
            nc.scalar.activation(lse[:], se_all[:], AF.Ln)
            nc.vector.tensor_tensor(lse[:], lse[:], mx_all[:], op=ALU.add)
            for t in range(L):
                lout = tail.tile([BL, T], dt.float32, tag="lout")
                nc.vector.tensor_scalar(lout[:], lg_all[:, t, :], lse[:, t:t + 1],
                                        None, op0=ALU.subtract)
                nc.sync.dma_start(
                    out_d.ap().rearrange("(b l) c -> b l c", l=L)[:, t, :], lout[:])
    nc.finalize()
    return nc


S2_SCALE = 1.0  # patched at build time (bn2 scale); module-level for closure use


def kernel(**inputs):
    global S2_SCALE
    import concourse.bass_utils as bass_utils

    tokens = np.asarray(inputs["tokens"])
    w2v = np.asarray(inputs["w2v"], np.float32)
    bn1 = np.asarray(inputs["bn1"], np.float32)
    bn2 = np.asarray(inputs["bn2"], np.float32)
    s1 = float(bn1[0] / np.sqrt(bn1[3] + BN_EPS))
    t1 = float(bn1[1] - bn1[2] * s1)
    s2 = float(bn2[0] / np.sqrt(bn2[3] + BN_EPS))
    t2 = float(bn2[1] - bn2[2] * s2)
    S2_SCALE = s2

    f32 = lambda k: np.asarray(inputs[k], np.float32)
    bft = lambda a: np.ascontiguousarray(np.asarray(a, np.float32).T).astype(BF16)
    enc_bih, enc_bhh = f32("enc_bih"), f32("enc_bhh")
    dec_bih, dec_bhh = f32("dec_bih"), f32("dec_bhh")
    egib = np.concatenate([enc_bih[:H] + enc_bhh[:H], enc_bih[H:2 * H] + enc_bhh[H:2 * H],
                           enc_bih[2 * H:]])
    dgib = np.concatenate([dec_bih[:H] + dec_bhh[:H], dec_bih[H:2 * H] + dec_bhh[H:2 * H],
                           dec_bih[2 * H:]])[None, :]
    out_W = f32("out_W")
    outWTs = np.ascontiguousarray((s1 * out_W).T).astype(BF16)
    lgb = (f32("out_b") + t1 * out_W.sum(axis=1))[None, :]
    combb = (f32("comb_b") + t2 / s2)[None, :]
    comb_W = f32("comb_W")

    # encoder Wih with bias folded as last row
    encWihT = np.zeros((301, G3), np.float32)
    encWihT[:300] = f32("enc_Wih").T
    encWihT[300] = egib

    # dec_emb rows 0..127 renormed (host)
    em = f32("dec_emb")[:128]
    emn = np.linalg.norm(em, axis=1, keepdims=True)
    embf = em * np.minimum(1.0, MAXN2 / (emn + 1e-7))
    # SOS embedding renormed, transposed, broadcast
    sos = f32("dec_emb")[T]
    sos = sos * min(1.0, MAXN2 / (np.linalg.norm(sos) + 1e-7))
    embT0 = np.broadcast_to(sos.reshape(2, 128).T[:, :, None], (128, 2, BL))

    ident = np.eye(128, dtype=np.float32)
    istk = np.zeros((128, BL), np.float32)
    istk[np.arange(128), np.arange(128) % BL] = 1.0
    istkb = np.tile(istk, (1, MT))

    common = {
        "encWihT": encWihT.astype(BF16), "encWhhT": bft(inputs["enc_Whh"]),
        "decWihT": bft(inputs["dec_Wih"]), "decWhhT": bft(inputs["dec_Whh"]),
        "combWembT": np.ascontiguousarray(comb_W[:, :D].T).astype(BF16),
        "combWappT": np.ascontiguousarray(comb_W[:, D:].T).astype(BF16),
        "outWTs": outWTs, "attnWT": bft(inputs["attn_W"]),
        "embf": np.ascontiguousarray(embf).astype(BF16),
        "embT0": np.ascontiguousarray(embT0.reshape(128, 2 * BL)).astype(BF16),
        "ident": ident, "istkb": istkb.astype(BF16),
        "ebhn": np.ascontiguousarray(enc_bhh[2 * H:][None, :]).astype(BF16),
        "dgib": np.ascontiguousarray(dgib).astype(BF16),
        "dbhn": np.ascontiguousarray(dec_bhh[2 * H:][None, :]).astype(BF16),
        "combb": np.ascontiguousarray(combb).astype(BF16),
        "attnb": np.ascontiguousarray(f32("attn_b")[None, :]).astype(BF16),
        "lgb": np.ascontiguousarray(lgb).astype(BF16),
    }
    in_maps = []
    for c in range(NC):
        tok = tokens[c * BL:(c + 1) * BL].astype(np.int64)        # (64,25)
        xg = w2v[tok]                                             # (64,25,300)
        nrm = np.linalg.norm(xg, axis=-1, keepdims=True)
        xg = xg * np.minimum(1.0, MAXN1 / (nrm + 1e-7))
        xTb = np.zeros((301, L * BL), np.float32)
        xTb[:300] = xg.transpose(2, 1, 0).reshape(E, L * BL)      # col = l*64+b
        xTb[300] = 1.0
        m = dict(common)
        m["xTb"] = xTb.astype(BF16)
        in_maps.append(m)

    nc = build_nc()
    trace = bool(int(os.environ.get("KERNEL_TRACE", "0")))
    res = bass_utils.run_bass_kernel_spmd(nc, in_maps, core_ids=list(range(NC)),
                                          trace=trace)
    if trace and res.exec_time_ns is not None:
        print(f"HW exec time: {res.exec_time_ns} ns", flush=True)
        print("trace:", res.instructions_and_trace[1] if res.instructions_and_trace else None,
              flush=True)
    out = np.concatenate([res.results[c]["out"] for c in range(NC)], axis=0)
    return out.astype(np.float32)


if __name__ == "__main__":
    pass


# revision 25
# speedup vs baseline: 2.5396x; 1.1345x over previous
"""Trainium2 Bass kernel for nn_Attention_72670846649042.

GRU encoder + greedy attention decoder, B=512,L=25,H=1024,D=256,T=128,E=300.
Sharding: data-parallel over batch, 64 rows/core on 8 cores, no collectives.

v3 design (v1 baseline 2.80 ms, v2 1.40 ms):
 - No DMA transposes: PE 128x128 transposes of folded [128,128] slices of the
   fp32 state produce two hT k-tiles per instruction. Keeps HAM warm.
 - Folded layout: hidden-halves at partitions 0:64 / 64:128; M=64 matmuls are
   column-packed in pairs with the pair ADJACENT in the PE queue (concurrent
   col groups), halving PE passes; DVE gate math runs at 128 lanes.
 - Encoder input projection inlined into the scan (bias as a ones-row of xT).
 - comb_W (applied part) folded into enc_out once (encC): the attention
   einsum directly produces the comb output.
 - Gate chains chunked into 256-col halves: transposes / state copies /
   next-step matmuls (ktile order [0,4,1,5,2,6,3,7]) start after half 1.
 - Activation tables: encoder {sigmoid,tanh}, decoder {exp,tanh,relu}
   (sigmoid via tanh identity), ln deferred to one batched tail.
"""
import os
import numpy as np
import ml_dtypes

B, L, V, E, H, D, T = 512, 25, 50000, 300, 1024, 256, 128
NC = 8
BL = B // NC          # 64 local batch
G3 = 3 * H            # 3072
KH = H // 128         # 8 hidden ktiles
MT = 13               # l-pair tiles for attention (25 -> 13 pairs, last padded)
MAXN1, MAXN2, BN_EPS = 10.0, 1.0, 1e-5
EK = (128, 128, 45)   # xT/encWih ktile rows (300 rows + 1 ones row)
KORD = (0, 4, 1, 5, 2, 6, 3, 7)   # ktile order gated by chunk-half readiness
BF16 = ml_dtypes.bfloat16

LINEARIZE = False


def build_nc():
    import concourse.bass as bass
    import concourse.tile as tile
    from concourse import bacc, mybir
    from contextlib import ExitStack

    dt = mybir.dt
    AF = mybir.ActivationFunctionType
    ALU = mybir.AluOpType
    AX = mybir.AxisListType

    nc = bacc.Bacc("TRN2", target_bir_lowering=False, debug=False)

    # ---- dram parameters ----
    xTb_d = nc.declare_dram_parameter("xTb", [301, L * BL], dt.bfloat16, isOutput=False)
    encWihT_d = nc.declare_dram_parameter("encWihT", [301, G3], dt.bfloat16, isOutput=False)
    encWhhT_d = nc.declare_dram_parameter("encWhhT", [H, G3], dt.bfloat16, isOutput=False)
    decWihT_d = nc.declare_dram_parameter("decWihT", [H, G3], dt.bfloat16, isOutput=False)
    decWhhT_d = nc.declare_dram_parameter("decWhhT", [H, G3], dt.bfloat16, isOutput=False)
    combWappT_d = nc.declare_dram_parameter("combWappT", [H, H], dt.bfloat16, isOutput=False)
    outWTs_d = nc.declare_dram_parameter("outWTs", [H, T], dt.bfloat16, isOutput=False)
    attnWT_d = nc.declare_dram_parameter("attnWT", [H, L], dt.bfloat16, isOutput=False)
    EA_d = nc.declare_dram_parameter("EA", [128, L], dt.bfloat16, isOutput=False)
    EC_d = nc.declare_dram_parameter("EC", [128, H], dt.bfloat16, isOutput=False)
    attnb0_d = nc.declare_dram_parameter("attnb0", [1, L], dt.bfloat16, isOutput=False)
    combb0_d = nc.declare_dram_parameter("combb0", [1, H], dt.bfloat16, isOutput=False)
    ident_d = nc.declare_dram_parameter("ident", [128, 128], dt.float32, isOutput=False)
    istkb_d = nc.declare_dram_parameter("istkb", [128, MT * BL], dt.bfloat16, isOutput=False)
    ebhn_d = nc.declare_dram_parameter("ebhn", [1, H], dt.bfloat16, isOutput=False)
    dgib_d = nc.declare_dram_parameter("dgib", [1, G3], dt.bfloat16, isOutput=False)
    dbhn_d = nc.declare_dram_parameter("dbhn", [1, H], dt.bfloat16, isOutput=False)
    combb_d = nc.declare_dram_parameter("combb", [1, H], dt.bfloat16, isOutput=False)
    attnb_d = nc.declare_dram_parameter("attnb", [1, L], dt.bfloat16, isOutput=False)
    lgb_d = nc.declare_dram_parameter("lgb", [1, T], dt.bfloat16, isOutput=False)
    out_d = nc.declare_dram_parameter("out", [BL * L, T], dt.float32, isOutput=True)

    with tile.TileContext(nc, linearize=LINEARIZE) as tc, ExitStack() as ctx:
        shared = ctx.enter_context(tc.tile_pool(name="shared", bufs=1))
        decw = ctx.enter_context(tc.tile_pool(name="decw", bufs=1))

        ident = shared.tile([128, 128], dt.float32, tag="ident")
        nc.sync.dma_start(ident[:], ident_d.ap())
        IstkB = shared.tile([128, MT, BL], dt.bfloat16, tag="IstkB")
        nc.sync.dma_start(IstkB[:], istkb_d.ap())
        EA = shared.tile([128, L], dt.bfloat16, tag="EA")
        nc.sync.dma_start(EA[:], EA_d.ap())
        EC = shared.tile([128, H], dt.bfloat16, tag="EC")
        nc.sync.dma_start(EC[:], EC_d.ap())
        attnWT = shared.tile([128, KH, L], dt.bfloat16, tag="attnWT")
        nc.sync.dma_start(attnWT[:], attnWT_d.ap().rearrange("(k p) n -> p k n", p=128))
        ones_sb = shared.tile([1, 128], dt.bfloat16, tag="ones_sb")
        nc.vector.memset(ones_sb[:], 1.0)
        attnb0_r = shared.tile([1, L], dt.bfloat16, tag="attnb0_r")
        nc.sync.dma_start(attnb0_r[:], attnb0_d.ap())
        combb0_r = shared.tile([1, H], dt.bfloat16, tag="combb0_r")
        nc.sync.dma_start(combb0_r[:], combb0_d.ap())

        ebhn_r = shared.tile([1, H], dt.bfloat16, tag="ebhn_r")
        nc.sync.dma_start(ebhn_r[:], ebhn_d.ap())
        dgib_r = shared.tile([1, G3], dt.bfloat16, tag="dgib_r")
        nc.sync.dma_start(dgib_r[:], dgib_d.ap())
        dbhn_r = shared.tile([1, H], dt.bfloat16, tag="dbhn_r")
        nc.sync.dma_start(dbhn_r[:], dbhn_d.ap())
        combb_r = shared.tile([1, H], dt.bfloat16, tag="combb_r")
        nc.sync.dma_start(combb_r[:], combb_d.ap())
        attnb_r = shared.tile([1, L], dt.bfloat16, tag="attnb_r")
        nc.sync.dma_start(attnb_r[:], attnb_d.ap())
        lgb_r = shared.tile([1, T], dt.bfloat16, tag="lgb_r")
        nc.sync.dma_start(lgb_r[:], lgb_d.ap())

        hA = shared.tile([128, 512], dt.float32, tag="hA")
        hB = shared.tile([128, 512], dt.float32, tag="hB")
        nc.vector.memset(hA[:], 0.0)
        h_tiles = [hA, hB]

        se_all = shared.tile([BL, L], dt.float32, tag="se_all")

        # decoder weights: decWhhT prefetched during encoder; rest at encC.
        decWhhT = decw.tile([128, KH, G3], dt.bfloat16, tag="decWhhT")
        nc.sync.dma_start(decWhhT[:], decWhhT_d.ap().rearrange("(k p) n -> p k n", p=128))
        outWTs = decw.tile([128, KH, T], dt.bfloat16, tag="outWTs")
        nc.sync.dma_start(outWTs[:], outWTs_d.ap().rearrange("(k p) n -> p k n", p=128))

        # encoder output history, transposed: [h-slice part, k, l, b], l=25 + pad
        enc_outT = decw.tile([128, KH, 26, BL], dt.bfloat16, tag="enc_outT")
        nc.vector.memset(enc_outT[:, :, 25, :], 0.0)

        def transp_half(hsrc, tp, c):
            # hsrc folded fp32 [128,512]; chunk c covers k-tiles {2c, 2c+1}
            # (partitions 0:64) and {2c+4, 2c+5} (partitions 64:128).
            for f in (2 * c, 2 * c + 1):
                nc.tensor.transpose(tp[:, f, :, :], hsrc[:, f * 128:(f + 1) * 128],
                                    ident[:])

        def copy_half(dst_kslices, tp, c):
            # dst view [128, hc, f(2), b] for f in {2c, 2c+1}
            nc.vector.tensor_copy(dst_kslices,
                                  tp[:, 2 * c:2 * c + 2, :, :].rearrange(
                                      "p f hc b -> p hc f b"))

        # =======================================================
        # Phase 1: encoder scan (gi inlined; 25 steps)
        # =======================================================
        with tc.tile_pool(name="encw", bufs=1) as encw, \
             tc.tile_pool(name="egps", bufs=2, space="PSUM") as egps, \
             tc.tile_pool(name="egp1", bufs=1, space="PSUM") as egp1, \
             tc.tile_pool(name="tpp", bufs=1, space="PSUM") as tpp, \
             tc.tile_pool(name="ework", bufs=2) as ework:
            xT = encw.tile([128, 3, L * BL], dt.bfloat16, tag="xT")
            nc.sync.dma_start(xT[:, 0, :], xTb_d.ap()[0:128, :])
            nc.sync.dma_start(xT[:, 1, :], xTb_d.ap()[128:256, :])
            nc.sync.dma_start(xT[0:45, 2, :], xTb_d.ap()[256:301, :])
            eWih = encw.tile([128, 3, G3], dt.bfloat16, tag="eWih")
            nc.sync.dma_start(eWih[:, 0, :], encWihT_d.ap()[0:128, :])
            nc.sync.dma_start(eWih[:, 1, :], encWihT_d.ap()[128:256, :])
            nc.sync.dma_start(eWih[0:45, 2, :], encWihT_d.ap()[256:301, :])
            eWhh = encw.tile([128, KH, G3], dt.bfloat16, tag="eWhh")
            nc.sync.dma_start(eWhh[:], encWhhT_d.ap().rearrange("(k p) n -> p k n", p=128))

            def emit_gi(t, ps_r, ps_z, ps_ngi, rz_stop):
                for g, bank, stp in ((0, ps_r, rz_stop), (2, ps_ngi, True),
                                     (1, ps_z, rz_stop)):
                    for kt in range(3):
                        for hc in range(2):
                            co = g * H + hc * 512
                            nc.tensor.matmul(
                                bank[hc * 64:(hc + 1) * 64, :],
                                xT[0:EK[kt], kt, t * BL:(t + 1) * BL],
                                eWih[0:EK[kt], kt, co:co + 512],
                                start=(kt == 0), stop=(stp and kt == 2))

            def alloc_banks():
                return (egps.tile([128, 512], dt.float32, name="ps_r", tag="r"),
                        egps.tile([128, 512], dt.float32, name="ps_z", tag="z"),
                        egps.tile([128, 512], dt.float32, name="ps_ngi", tag="ngi"))

            banks = {}
            banks[0] = alloc_banks()
            emit_gi(0, *banks[0], rz_stop=True)
            for t in range(L):
                ps_r, ps_z, ps_ngi = banks.pop(t)
                ps_ngh = egp1.tile([128, 512], dt.float32, tag="ngh")
                # gh matmuls (skip at t=0: h=0); bank order r, ngh, z so the
                # r/ngh-dependent gate chain starts earliest.
                if t > 0:
                    for g, bank, st in ((0, ps_r, False), (2, ps_ngh, True),
                                        (1, ps_z, False)):
                        for ki, k in enumerate(KORD):
                            for hc in range(2):
                                co = g * H + hc * 512
                                nc.tensor.matmul(
                                    bank[hc * 64:(hc + 1) * 64, :],
                                    enc_outT[:, k, t - 1, :],
                                    eWhh[:, k, co:co + 512],
                                    start=(st and ki == 0),
                                    stop=(not st and ki == KH - 1))
                for hc in range(2):
                    nc.tensor.matmul(ps_ngh[hc * 64:(hc + 1) * 64, :],
                                     ones_sb[0:1, 0:BL],
                                     ebhn_r[0:1, hc * 512:hc * 512 + 512],
                                     start=(t == 0), stop=True)
                # next step's gi (fills PE while this step's gate chain runs)
                if t + 1 < L:
                    banks[t + 1] = alloc_banks()
                    emit_gi(t + 1, *banks[t + 1], rz_stop=False)
                # ---- gates, chunked in 256-col halves ----
                hprev = h_tiles[t % 2]
                hnew = h_tiles[(t + 1) % 2]
                r_s = ework.tile([128, 512], dt.bfloat16, tag="r_s")
                z_s = ework.tile([128, 512], dt.bfloat16, tag="z_s")
                nt = ework.tile([128, 512], dt.float32, tag="nt", bufs=1)
                n_s = ework.tile([128, 512], dt.float32, tag="n_s", bufs=1)
                t4 = ework.tile([128, 512], dt.float32, tag="t4", bufs=1)
                tp = tpp.tile([128, 4, 2, BL], dt.float32, tag="tp")
                eo_view = enc_outT[:, :, t, :].rearrange("p (hc f) b -> p hc f b", hc=2)
                for c in range(2):
                    sl = slice(c * 256, (c + 1) * 256)
                    nc.scalar.activation(r_s[:, sl], ps_r[:, sl], AF.Sigmoid)
                    nc.scalar.activation(z_s[:, sl], ps_z[:, sl], AF.Sigmoid)
                    nc.vector.tensor_tensor(nt[:, sl], ps_ngh[:, sl], r_s[:, sl],
                                            op=ALU.mult)
                    nc.vector.tensor_tensor(nt[:, sl], nt[:, sl], ps_ngi[:, sl],
                                            op=ALU.add)
                    nc.scalar.activation(n_s[:, sl], nt[:, sl], AF.Tanh)
                    nc.vector.tensor_tensor(t4[:, sl], hprev[:, sl], n_s[:, sl],
                                            op=ALU.subtract)
                    nc.vector.tensor_tensor(t4[:, sl], t4[:, sl], z_s[:, sl],
                                            op=ALU.mult)
                    nc.vector.tensor_tensor(hnew[:, sl], n_s[:, sl], t4[:, sl],
                                            op=ALU.add)
                    transp_half(hnew, tp, c)
                    copy_half(eo_view[:, :, 2 * c:2 * c + 2, :], tp, c)

        # =======================================================
        # Phase 2: encC = enc_out @ combW_app   (+ load decoder weights)
        # =======================================================
        decw2 = ctx.enter_context(tc.tile_pool(name="decw2", bufs=1))
        decWihT = decw2.tile([128, KH, G3], dt.bfloat16, tag="decWihT")
        nc.sync.dma_start(decWihT[:], decWihT_d.ap().rearrange("(k p) n -> p k n", p=128))
        encC = decw2.tile([128, MT, H], dt.bfloat16, tag="encC")
        lg_all = decw2.tile([BL, L, T], dt.float32, tag="lg_all")
        hTt = decw2.tile([128, KH, BL], dt.bfloat16, tag="hTt")
        oTt = decw2.tile([128, KH, BL], dt.bfloat16, tag="oTt")
        with tc.tile_pool(name="ccw", bufs=1) as ccw, \
             tc.tile_pool(name="ccps", bufs=4, space="PSUM") as ccps:
            combWappT = ccw.tile([128, KH, H], dt.bfloat16, tag="combWappT")
            nc.sync.dma_start(combWappT[:],
                              combWappT_d.ap().rearrange("(k p) n -> p k n", p=128))
            for m in range(MT):
                for nch in range(2):
                    ps = ccps.tile([128, 512], dt.float32, tag="cc")
                    for k in range(KH):
                        nc.tensor.matmul(
                            ps[:], enc_outT[:, k, 2 * m:2 * m + 2, :],
                            combWappT[:, k, nch * 512:(nch + 1) * 512],
                            start=(k == 0), stop=(k == KH - 1))
                    nc.vector.tensor_copy(encC[:, m, nch * 512:(nch + 1) * 512], ps[:])
        nc.vector.tensor_copy(hTt[:], enc_outT[:, :, 24, :])

        # =======================================================
        # Phase 3: decoder (25 steps)
        # =======================================================
        with tc.tile_pool(name="dgps", bufs=1, space="PSUM") as dgps, \
             tc.tile_pool(name="dops", bufs=1, space="PSUM") as dops, \
             tc.tile_pool(name="tpp2", bufs=1, space="PSUM") as tpp2, \
             tc.tile_pool(name="mscp", bufs=1, space="PSUM") as mscp, \
             tc.tile_pool(name="lgps", bufs=1, space="PSUM") as lgps, \
             tc.tile_pool(name="dwork", bufs=2) as dwork:
            oh_prev = None
            for t in range(L):
                # ---- gh matmuls first: they only need hTt, so they fill the
                # PE while step t-1's argmax tail runs on DVE/ACT ----
                ps_r = dgps.tile([128, 512], dt.float32, tag="r")
                ps_z = dgps.tile([128, 512], dt.float32, tag="z")
                ps_ngh = dgps.tile([128, 512], dt.float32, tag="ngh")
                ps_ngi = dgps.tile([128, 512], dt.float32, tag="ngi")
                for g, bank in ((0, ps_r), (1, ps_z), (2, ps_ngh)):
                    for ki, k in enumerate(KORD):
                        for hc in range(2):
                            co = g * H + hc * 512
                            nc.tensor.matmul(bank[hc * 64:(hc + 1) * 64, :],
                                             hTt[:, k, :], decWhhT[:, k, co:co + 512],
                                             start=(ki == 0), stop=False)
                for hc in range(2):
                    nc.tensor.matmul(ps_ngh[hc * 64:(hc + 1) * 64, :],
                                     ones_sb[0:1, 0:BL],
                                     dbhn_r[0:1, hc * 512:hc * 512 + 512],
                                     start=False, stop=True)
                # ---- argmax token of step t-1 -> ohT ----
                if t > 0:
                    tp0 = tpp2.tile([128, 4, 2, BL], dt.float32, tag="tp")
                    nc.tensor.transpose(tp0[:, 0, :, :], oh_prev[:], ident[0:BL, :])
                    ohT = dwork.tile([128, BL], dt.bfloat16, tag="ohT")
                    nc.vector.tensor_copy(ohT[:], tp0[:, 0, 0, :])
                # ---- attention scores -> misc[0:64, 128:153] ----
                misc = mscp.tile([128, 512], dt.float32, tag="misc")
                sc = misc[0:BL, 128:128 + L]
                if t > 0:
                    nc.tensor.matmul(sc, ohT[:], EA[:], start=True, stop=False)
                for ki, k in enumerate(KORD):
                    nc.tensor.matmul(sc, hTt[:, k, :], attnWT[:, k, :],
                                     start=(t == 0 and ki == 0), stop=False)
                nc.tensor.matmul(sc, ones_sb[0:1, 0:BL],
                                 attnb_r[:] if t > 0 else attnb0_r[:],
                                 start=False, stop=True)
                # ---- softmax over scores (no max shift: scores are small) ----
                aw = dwork.tile([BL, L], dt.float32, tag="aw")
                sume = dwork.tile([BL, 1], dt.float32, tag="sume")
                nc.scalar.activation(aw[:], sc, AF.Exp, accum_out=sume[:])
                rs = dwork.tile([BL, 1], dt.float32, tag="rs")
                nc.vector.reciprocal(rs[:], sume[:])
                rs2 = dwork.tile([128, 1], dt.float32, tag="rs2")
                nc.vector.tensor_copy(rs2[0:BL, :], rs[:])
                nc.vector.tensor_copy(rs2[BL:128, :], rs[:])
                # awn[q, p] = aw[b, 2p + (q>=64)] / sum  (l=25 slot zero)
                awn = dwork.tile([128, MT], dt.float32, tag="awn")
                nc.vector.tensor_copy(awn[0:BL, :], aw[:, 0:25:2])
                nc.vector.tensor_copy(awn[BL:128, 0:12], aw[:, 1:25:2])
                nc.vector.memset(awn[BL:128, 12:13], 0.0)
                nc.vector.tensor_scalar(awn[:], awn[:], rs2[:], None, op0=ALU.mult)
                dgs = dwork.tile([128, MT, BL], dt.bfloat16, tag="dgs", bufs=1)
                nc.vector.tensor_tensor(dgs[:], IstkB[:],
                                        awn[:].broadcast_to((128, MT, BL)),
                                        op=ALU.mult)
                # ---- o = emb-part(via EC) + einsum(aw, encC) + combb ----
                ps_o = dops.tile([128, 512], dt.float32, tag="o")
                if t > 0:
                    for hc in range(2):
                        nc.tensor.matmul(ps_o[hc * 64:(hc + 1) * 64, :], ohT[:],
                                         EC[:, hc * 512:(hc + 1) * 512],
                                         start=True, stop=False)
                for hc in range(2):
                    nc.tensor.matmul(ps_o[hc * 64:(hc + 1) * 64, :],
                                     ones_sb[0:1, 0:BL],
                                     combb_r[0:1, hc * 512:hc * 512 + 512]
                                     if t > 0 else
                                     combb0_r[0:1, hc * 512:hc * 512 + 512],
                                     start=(t == 0), stop=False)
                for p in range(MT):
                    for hc in range(2):
                        nc.tensor.matmul(ps_o[hc * 64:(hc + 1) * 64, :],
                                         dgs[:, p, :],
                                         encC[:, p, hc * 512:(hc + 1) * 512],
                                         start=False, stop=(p == MT - 1))
                obf = dwork.tile([128, 512], dt.float32, tag="obf")
                nc.scalar.activation(obf[:], ps_o[:], AF.Relu, scale=S2_SCALE)
                # ---- oT ----
                tp = tpp2.tile([128, 4, 2, BL], dt.float32, tag="tp")
                for c in range(2):
                    transp_half(obf, tp, c)
                nc.vector.tensor_copy(
                    oTt[:].rearrange("p (hc f) b -> p hc f b", hc=2),
                    tp[:].rearrange("p f hc b -> p hc f b"))
                # ---- gi matmuls from oT; bank order r, ngi, z ----
                for g, bank in ((0, ps_r), (2, ps_ngi), (1, ps_z)):
                    for ki, k in enumerate(KORD):
                        for hc in range(2):
                            co = g * H + hc * 512
                            nc.tensor.matmul(bank[hc * 64:(hc + 1) * 64, :],
                                             oTt[:, k, :], decWihT[:, k, co:co + 512],
                                             start=(g == 2 and ki == 0), stop=False)
                    for hc in range(2):
                        co = g * H + hc * 512
                        nc.tensor.matmul(bank[hc * 64:(hc + 1) * 64, :],
                                         ones_sb[0:1, 0:BL],
                                         dgib_r[0:1, co:co + 512],
                                         start=False, stop=True)
                # ---- gates (sigma via tanh), chunked halves ----
                hprev = h_tiles[(L + t) % 2]
                hnew = h_tiles[(L + t + 1) % 2]
                r_s = dwork.tile([128, 512], dt.bfloat16, tag="r_s")
                z_s = dwork.tile([128, 512], dt.bfloat16, tag="z_s")
                nt = dwork.tile([128, 512], dt.float32, tag="nt", bufs=1)
                n_s = dwork.tile([128, 512], dt.float32, tag="n_s", bufs=1)
                t4 = dwork.tile([128, 512], dt.float32, tag="t4", bufs=1)
                tp2 = tpp2.tile([128, 4, 2, BL], dt.float32, tag="tp")
                lg = lgps.tile([BL, T], dt.float32, tag="lg")
                hT_view = hTt[:].rearrange("p (hc f) b -> p hc f b", hc=2)
                for c in range(2):
                    sl = slice(c * 256, (c + 1) * 256)
                    nc.scalar.activation(r_s[:, sl], ps_r[:, sl], AF.Tanh, scale=0.5)
                    nc.vector.tensor_scalar(r_s[:, sl], r_s[:, sl], 0.5, 0.5,
                                            op0=ALU.mult, op1=ALU.add)
                    nc.scalar.activation(z_s[:, sl], ps_z[:, sl], AF.Tanh, scale=0.5)
                    nc.vector.tensor_scalar(z_s[:, sl], z_s[:, sl], 0.5, 0.5,
                                            op0=ALU.mult, op1=ALU.add)
                    nc.vector.tensor_tensor(nt[:, sl], ps_ngh[:, sl], r_s[:, sl],
                                            op=ALU.mult)
                    nc.vector.tensor_tensor(nt[:, sl], nt[:, sl], ps_ngi[:, sl],
                                            op=ALU.add)
                    nc.scalar.activation(n_s[:, sl], nt[:, sl], AF.Tanh)
                    nc.vector.tensor_tensor(t4[:, sl], hprev[:, sl], n_s[:, sl],
                                            op=ALU.subtract)
                    nc.vector.tensor_tensor(t4[:, sl], t4[:, sl], z_s[:, sl],
                                            op=ALU.mult)
                    nc.vector.tensor_tensor(hnew[:, sl], n_s[:, sl], t4[:, sl],
                                            op=ALU.add)
                    transp_half(hnew, tp2, c)
                    copy_half(hT_view[:, :, 2 * c:2 * c + 2, :], tp2, c)
                    # logits for the k-tiles this half provides
                    for ki, k in enumerate(KORD[4 * c:4 * c + 4]):
                        nc.tensor.matmul(lg[:], hTt[:, k, :], outWTs[:, k, :],
                                         start=(c == 0 and ki == 0), stop=False)
                nc.tensor.matmul(lg[:], ones_sb[0:1, 0:BL], lgb_r[:],
                                 start=False, stop=True)
                nc.scalar.copy(lg_all[:, t, :], lg[:])
                ex = dwork.tile([BL, T], dt.float32, tag="ex")
                nc.scalar.activation(ex[:], lg[:], AF.Exp,
                                     accum_out=se_all[:, t:t + 1])
                # ---- argmax onehot (transposed next iteration) ----
                if t < L - 1:
                    mx2 = dwork.tile([BL, 1], dt.float32, tag="mx2")
                    nc.vector.tensor_reduce(mx2[:], lg[:], axis=AX.X, op=ALU.max)
                    oh_prev = dwork.tile([BL, T], dt.float32, tag="oh")
                    nc.vector.tensor_scalar(oh_prev[:], lg[:], mx2[:], None,
                                            op0=ALU.is_equal)

        # =======================================================
        # Phase 4: log-softmax tail
        # =======================================================
        with tc.tile_pool(name="tail", bufs=2) as tail:
            lse = tail.tile([BL, L], dt.float32, tag="lse", bufs=1)
            nc.scalar.activation(lse[:], se_all[:], AF.Ln)
            for t in range(L):
                lout = tail.tile([BL, T], dt.float32, tag="lout")
                nc.vector.tensor_scalar(lout[:], lg_all[:, t, :], lse[:, t:t + 1],
                                        None, op0=ALU.subtract)
                nc.sync.dma_start(
                    out_d.ap().rearrange("(b l) c -> b l c", l=L)[:, t, :], lout[:])
    nc.finalize()
    return nc


S2_SCALE = 1.0  # patched at build time (bn2 scale); module-level for closure use


def kernel(**inputs):
    global S2_SCALE
    import concourse.bass_utils as bass_utils

    tokens = np.asarray(inputs["tokens"])
    w2v = np.asarray(inputs["w2v"], np.float32)
    bn1 = np.asarray(inputs["bn1"], np.float32)
    bn2 = np.asarray(inputs["bn2"], np.float32)
    s1 = float(bn1[0] / np.sqrt(bn1[3] + BN_EPS))
    t1 = float(bn1[1] - bn1[2] * s1)
    s2 = float(bn2[0] / np.sqrt(bn2[3] + BN_EPS))
    t2 = float(bn2[1] - bn2[2] * s2)
    S2_SCALE = s2

    f32 = lambda k: np.asarray(inputs[k], np.float32)
    bft = lambda a: np.ascontiguousarray(np.asarray(a, np.float32).T).astype(BF16)
    enc_bih, enc_bhh = f32("enc_bih"), f32("enc_bhh")
    dec_bih, dec_bhh = f32("dec_bih"), f32("dec_bhh")
    egib = np.concatenate([enc_bih[:H] + enc_bhh[:H], enc_bih[H:2 * H] + enc_bhh[H:2 * H],
                           enc_bih[2 * H:]])
    dgib = np.concatenate([dec_bih[:H] + dec_bhh[:H], dec_bih[H:2 * H] + dec_bhh[H:2 * H],
                           dec_bih[2 * H:]])[None, :]
    out_W = f32("out_W")
    outWTs = np.ascontiguousarray((s1 * out_W).T).astype(BF16)
    lgb = (f32("out_b") + t1 * out_W.sum(axis=1))[None, :]
    combb = (f32("comb_b") + t2 / s2)[None, :]
    comb_W = f32("comb_W")

    # encoder Wih with bias folded as last row
    encWihT = np.zeros((301, G3), np.float32)
    encWihT[:300] = f32("enc_Wih").T
    encWihT[300] = egib

    # dec_emb rows 0..127 renormed (host); fold emb@attnW_emb / emb@combW_emb
    em = f32("dec_emb")[:128]
    emn = np.linalg.norm(em, axis=1, keepdims=True)
    embf = em * np.minimum(1.0, MAXN2 / (emn + 1e-7))
    attn_W = f32("attn_W")
    EA = embf @ attn_W[:, :D].T                       # (128, L)
    EC = embf @ comb_W[:, :D].T                       # (128, H)
    # SOS embedding renormed -> step-0 bias rows
    sos = f32("dec_emb")[T]
    sos = sos * min(1.0, MAXN2 / (np.linalg.norm(sos) + 1e-7))
    attnb = f32("attn_b")[None, :]
    attnb0 = attnb + (sos @ attn_W[:, :D].T)[None, :]
    combb0 = combb + (sos @ comb_W[:, :D].T)[None, :]

    ident = np.eye(128, dtype=np.float32)
    istk = np.zeros((128, BL), np.float32)
    istk[np.arange(128), np.arange(128) % BL] = 1.0
    istkb = np.tile(istk, (1, MT))

    common = {
        "encWihT": encWihT.astype(BF16), "encWhhT": bft(inputs["enc_Whh"]),
        "decWihT": bft(inputs["dec_Wih"]), "decWhhT": bft(inputs["dec_Whh"]),
        "combWappT": np.ascontiguousarray(comb_W[:, D:].T).astype(BF16),
        "outWTs": outWTs,
        "attnWT": np.ascontiguousarray(attn_W[:, D:].T).astype(BF16),
        "EA": np.ascontiguousarray(EA).astype(BF16),
        "EC": np.ascontiguousarray(EC).astype(BF16),
        "attnb0": np.ascontiguousarray(attnb0).astype(BF16),
        "combb0": np.ascontiguousarray(combb0).astype(BF16),
        "ident": ident, "istkb": istkb.astype(BF16),
        "ebhn": np.ascontiguousarray(enc_bhh[2 * H:][None, :]).astype(BF16),
        "dgib": np.ascontiguousarray(dgib).astype(BF16),
        "dbhn": np.ascontiguousarray(dec_bhh[2 * H:][None, :]).astype(BF16),
        "combb": np.ascontiguousarray(combb).astype(BF16),
        "attnb": np.ascontiguousarray(attnb).astype(BF16),
        "lgb": np.ascontiguousarray(lgb).astype(BF16),
    }
    in_maps = []
    for c in range(NC):
        tok = tokens[c * BL:(c + 1) * BL].astype(np.int64)        # (64,25)
        xg = w2v[tok]                                             # (64,25,300)
        nrm = np.linalg.norm(xg, axis=-1, keepdims=True)
        xg = xg * np.minimum(1.0, MAXN1 / (nrm + 1e-7))
        xTb = np.zeros((301, L * BL), np.float32)
        xTb[:300] = xg.transpose(2, 1, 0).reshape(E, L * BL)      # col = l*64+b
        xTb[300] = 1.0
        m = dict(common)
        m["xTb"] = xTb.astype(BF16)
        in_maps.append(m)

    nc = build_nc()
    trace = bool(int(os.environ.get("KERNEL_TRACE", "0")))
    res = bass_utils.run_bass_kernel_spmd(nc, in_maps, core_ids=list(range(NC)),
                                          trace=trace)
    if trace and res.exec_time_ns is not None:
        print(f"HW exec time: {res.exec_time_ns} ns", flush=True)
        print("trace:", res.instructions_and_trace[1] if res.instructions_and_trace else None,
              flush=True)
    out = np.concatenate([res.results[c]["out"] for c in range(NC)], axis=0)
    return out.astype(np.float32)


if __name__ == "__main__":
    pass


# revision 31
# speedup vs baseline: 2.6898x; 1.0591x over previous
"""Trainium2 Bass kernel for nn_Attention_72670846649042.

GRU encoder + greedy attention decoder, B=512,L=25,H=1024,D=256,T=128,E=300.
Sharding: data-parallel over batch, 64 rows/core on 8 cores, no collectives.

v3 design (v1 baseline 2.80 ms, v2 1.40 ms):
 - No DMA transposes: PE 128x128 transposes of folded [128,128] slices of the
   fp32 state produce two hT k-tiles per instruction. Keeps HAM warm.
 - Folded layout: hidden-halves at partitions 0:64 / 64:128; M=64 matmuls are
   column-packed in pairs with the pair ADJACENT in the PE queue (concurrent
   col groups), halving PE passes; DVE gate math runs at 128 lanes.
 - Encoder input projection inlined into the scan (bias as a ones-row of xT).
 - comb_W (applied part) folded into enc_out once (encC): the attention
   einsum directly produces the comb output.
 - Gate chains chunked into 256-col halves: transposes / state copies /
   next-step matmuls (ktile order [0,4,1,5,2,6,3,7]) start after half 1.
 - Activation tables: encoder {sigmoid,tanh}, decoder {exp,tanh,relu}
   (sigmoid via tanh identity), ln deferred to one batched tail.
"""
import os
import numpy as np
import ml_dtypes

B, L, V, E, H, D, T = 512, 25, 50000, 300, 1024, 256, 128
NC = 8
BL = B // NC          # 64 local batch
G3 = 3 * H            # 3072
KH = H // 128         # 8 hidden ktiles
MT = 13               # l-pair tiles for attention (25 -> 13 pairs, last padded)
MAXN1, MAXN2, BN_EPS = 10.0, 1.0, 1e-5
EK = (128, 128, 45)   # xT/encWih ktile rows (300 rows + 1 ones row)
KORD = (0, 4, 1, 5, 2, 6, 3, 7)   # ktile order gated by chunk-half readiness
BF16 = ml_dtypes.bfloat16

LINEARIZE = False


def build_nc():
    import concourse.bass as bass
    import concourse.tile as tile
    from concourse import bacc, mybir
    from contextlib import ExitStack

    dt = mybir.dt
    AF = mybir.ActivationFunctionType
    ALU = mybir.AluOpType
    AX = mybir.AxisListType

    nc = bacc.Bacc("TRN2", target_bir_lowering=False, debug=False)

    # ---- dram parameters ----
    xTb_d = nc.declare_dram_parameter("xTb", [301, L * BL], dt.bfloat16, isOutput=False)
    encWihT_d = nc.declare_dram_parameter("encWihT", [301, G3], dt.bfloat16, isOutput=False)
    encWhhT_d = nc.declare_dram_parameter("encWhhT", [H, G3], dt.bfloat16, isOutput=False)
    decWihT_d = nc.declare_dram_parameter("decWihT", [H, G3], dt.bfloat16, isOutput=False)
    decWhhT_d = nc.declare_dram_parameter("decWhhT", [H, G3], dt.bfloat16, isOutput=False)
    combWappT_d = nc.declare_dram_parameter("combWappT", [H, H], dt.bfloat16, isOutput=False)
    outWTs_d = nc.declare_dram_parameter("outWTs", [H, T], dt.bfloat16, isOutput=False)
    attnWT_d = nc.declare_dram_parameter("attnWT", [H, L], dt.bfloat16, isOutput=False)
    EA_d = nc.declare_dram_parameter("EA", [128, L], dt.bfloat16, isOutput=False)
    EC_d = nc.declare_dram_parameter("EC", [128, H], dt.bfloat16, isOutput=False)
    attnb0_d = nc.declare_dram_parameter("attnb0", [1, L], dt.bfloat16, isOutput=False)
    combb0_d = nc.declare_dram_parameter("combb0", [1, H], dt.bfloat16, isOutput=False)
    ident_d = nc.declare_dram_parameter("ident", [128, 128], dt.float32, isOutput=False)
    istkb_d = nc.declare_dram_parameter("istkb", [128, MT * BL], dt.bfloat16, isOutput=False)
    ebhn_d = nc.declare_dram_parameter("ebhn", [1, H], dt.bfloat16, isOutput=False)
    dgib_d = nc.declare_dram_parameter("dgib", [1, G3], dt.bfloat16, isOutput=False)
    dbhn_d = nc.declare_dram_parameter("dbhn", [1, H], dt.bfloat16, isOutput=False)
    combb_d = nc.declare_dram_parameter("combb", [1, H], dt.bfloat16, isOutput=False)
    attnb_d = nc.declare_dram_parameter("attnb", [1, L], dt.bfloat16, isOutput=False)
    lgb_d = nc.declare_dram_parameter("lgb", [1, T], dt.bfloat16, isOutput=False)
    out_d = nc.declare_dram_parameter("out", [BL * L, T], dt.float32, isOutput=True)

    with tile.TileContext(nc, linearize=LINEARIZE) as tc, ExitStack() as ctx:
        shared = ctx.enter_context(tc.tile_pool(name="shared", bufs=1))
        decw = ctx.enter_context(tc.tile_pool(name="decw", bufs=1))

        ident = shared.tile([128, 128], dt.float32, tag="ident")
        nc.sync.dma_start(ident[:], ident_d.ap())
        IstkB = shared.tile([128, MT, BL], dt.bfloat16, tag="IstkB")
        nc.sync.dma_start(IstkB[:], istkb_d.ap())
        EA = shared.tile([128, L], dt.bfloat16, tag="EA")
        nc.sync.dma_start(EA[:], EA_d.ap())
        EC = shared.tile([128, H], dt.bfloat16, tag="EC")
        nc.sync.dma_start(EC[:], EC_d.ap())
        attnWT = shared.tile([128, KH, L], dt.bfloat16, tag="attnWT")
        nc.sync.dma_start(attnWT[:], attnWT_d.ap().rearrange("(k p) n -> p k n", p=128))
        ones_sb = shared.tile([1, 128], dt.bfloat16, tag="ones_sb")
        nc.vector.memset(ones_sb[:], 1.0)
        attnb0_r = shared.tile([1, L], dt.bfloat16, tag="attnb0_r")
        nc.sync.dma_start(attnb0_r[:], attnb0_d.ap())
        combb0_r = shared.tile([1, H], dt.bfloat16, tag="combb0_r")
        nc.sync.dma_start(combb0_r[:], combb0_d.ap())

        ebhn_r = shared.tile([1, H], dt.bfloat16, tag="ebhn_r")
        nc.sync.dma_start(ebhn_r[:], ebhn_d.ap())
        dgib_r = shared.tile([1, G3], dt.bfloat16, tag="dgib_r")
        nc.sync.dma_start(dgib_r[:], dgib_d.ap())
        dbhn_r = shared.tile([1, H], dt.bfloat16, tag="dbhn_r")
        nc.sync.dma_start(dbhn_r[:], dbhn_d.ap())
        combb_r = shared.tile([1, H], dt.bfloat16, tag="combb_r")
        nc.sync.dma_start(combb_r[:], combb_d.ap())
        attnb_r = shared.tile([1, L], dt.bfloat16, tag="attnb_r")
        nc.sync.dma_start(attnb_r[:], attnb_d.ap())
        lgb_r = shared.tile([1, T], dt.bfloat16, tag="lgb_r")
        nc.sync.dma_start(lgb_r[:], lgb_d.ap())

        hA = shared.tile([128, 512], dt.float32, tag="hA")
        hB = shared.tile([128, 512], dt.float32, tag="hB")
        nc.vector.memset(hA[:], 0.0)
        h_tiles = [hA, hB]

        se_all = shared.tile([BL, L], dt.float32, tag="se_all")

        # decoder weights: decWhhT prefetched during encoder; rest at encC.
        decWhhT = decw.tile([128, KH, G3], dt.bfloat16, tag="decWhhT")
        nc.sync.dma_start(decWhhT[:], decWhhT_d.ap().rearrange("(k p) n -> p k n", p=128))
        outWTs = decw.tile([128, KH, T], dt.bfloat16, tag="outWTs")
        nc.sync.dma_start(outWTs[:], outWTs_d.ap().rearrange("(k p) n -> p k n", p=128))

        # encoder output history, transposed: [h-slice part, k, l, b], l=25 + pad
        enc_outT = decw.tile([128, KH, 26, BL], dt.bfloat16, tag="enc_outT")
        nc.vector.memset(enc_outT[:, :, 25, :], 0.0)

        def transp_half(hsrc, tp, c):
            # hsrc folded fp32 [128,512]; chunk c covers k-tiles {2c, 2c+1}
            # (partitions 0:64) and {2c+4, 2c+5} (partitions 64:128).
            for f in (2 * c, 2 * c + 1):
                nc.tensor.transpose(tp[:, f, :, :], hsrc[:, f * 128:(f + 1) * 128],
                                    ident[:])

        def copy_half(dst_kslices, tp, c):
            # dst view [128, hc, f(2), b] for f in {2c, 2c+1}
            nc.vector.tensor_copy(dst_kslices,
                                  tp[:, 2 * c:2 * c + 2, :, :].rearrange(
                                      "p f hc b -> p hc f b"))

        def warm(tp, f, src):
            # dummy PE transpose gated on `src`: keeps HAM from re-throttling
            # the PE during long DVE/ACT chain windows.
            nc.tensor.transpose(tp[:, f, :, :], src, ident[0:src.partition_size(), :])

        # =======================================================
        # Phase 1: encoder scan (gi inlined; 25 steps)
        # =======================================================
        with tc.tile_pool(name="encw", bufs=1) as encw, \
             tc.tile_pool(name="egps", bufs=2, space="PSUM") as egps, \
             tc.tile_pool(name="egp1", bufs=1, space="PSUM") as egp1, \
             tc.tile_pool(name="tpp", bufs=1, space="PSUM") as tpp, \
             tc.tile_pool(name="ework", bufs=2) as ework:
            xT = encw.tile([128, 3, L * BL], dt.bfloat16, tag="xT")
            nc.sync.dma_start(xT[:, 0, :], xTb_d.ap()[0:128, :])
            nc.sync.dma_start(xT[:, 1, :], xTb_d.ap()[128:256, :])
            nc.sync.dma_start(xT[0:45, 2, :], xTb_d.ap()[256:301, :])
            eWih = encw.tile([128, 3, G3], dt.bfloat16, tag="eWih")
            nc.sync.dma_start(eWih[:, 0, :], encWihT_d.ap()[0:128, :])
            nc.sync.dma_start(eWih[:, 1, :], encWihT_d.ap()[128:256, :])
            nc.sync.dma_start(eWih[0:45, 2, :], encWihT_d.ap()[256:301, :])
            eWhh = encw.tile([128, KH, G3], dt.bfloat16, tag="eWhh")
            nc.sync.dma_start(eWhh[:], encWhhT_d.ap().rearrange("(k p) n -> p k n", p=128))

            def emit_gi(t, ps_r, ps_z, ps_ngi, rz_stop):
                for g, bank, stp in ((0, ps_r, rz_stop), (2, ps_ngi, True),
                                     (1, ps_z, rz_stop)):
                    for kt in range(3):
                        for hc in range(2):
                            co = g * H + hc * 512
                            nc.tensor.matmul(
                                bank[hc * 64:(hc + 1) * 64, :],
                                xT[0:EK[kt], kt, t * BL:(t + 1) * BL],
                                eWih[0:EK[kt], kt, co:co + 512],
                                start=(kt == 0), stop=(stp and kt == 2))

            def alloc_banks():
                return (egps.tile([128, 512], dt.float32, name="ps_r", tag="r"),
                        egps.tile([128, 512], dt.float32, name="ps_z", tag="z"),
                        egps.tile([128, 512], dt.float32, name="ps_ngi", tag="ngi"))

            banks = {}
            banks[0] = alloc_banks()
            emit_gi(0, *banks[0], rz_stop=True)
            for t in range(L):
                ps_r, ps_z, ps_ngi = banks.pop(t)
                ps_ngh = egp1.tile([128, 512], dt.float32, tag="ngh")
                # gh matmuls (skip at t=0: h=0); bank order r, ngh, z so the
                # r/ngh-dependent gate chain starts earliest.
                if t > 0:
                    for g, bank, st in ((0, ps_r, False), (2, ps_ngh, True),
                                        (1, ps_z, False)):
                        for ki, k in enumerate(KORD):
                            for hc in range(2):
                                co = g * H + hc * 512
                                nc.tensor.matmul(
                                    bank[hc * 64:(hc + 1) * 64, :],
                                    enc_outT[:, k, t - 1, :],
                                    eWhh[:, k, co:co + 512],
                                    start=(st and ki == 0),
                                    stop=(not st and ki == KH - 1))
                for hc in range(2):
                    nc.tensor.matmul(ps_ngh[hc * 64:(hc + 1) * 64, :],
                                     ones_sb[0:1, 0:BL],
                                     ebhn_r[0:1, hc * 512:hc * 512 + 512],
                                     start=(t == 0), stop=True)
                # next step's gi (fills PE while this step's gate chain runs)
                if t + 1 < L:
                    banks[t + 1] = alloc_banks()
                    emit_gi(t + 1, *banks[t + 1], rz_stop=False)
                # ---- gates, chunked in 256-col halves ----
                hprev = h_tiles[t % 2]
                hnew = h_tiles[(t + 1) % 2]
                r_s = ework.tile([128, 512], dt.bfloat16, tag="r_s")
                z_s = ework.tile([128, 512], dt.bfloat16, tag="z_s")
                nt = ework.tile([128, 512], dt.float32, tag="nt", bufs=1)
                n_s = ework.tile([128, 512], dt.float32, tag="n_s", bufs=1)
                t4 = ework.tile([128, 512], dt.float32, tag="t4", bufs=1)
                tp = tpp.tile([128, 4, 2, BL], dt.float32, tag="tp")
                eo_view = enc_outT[:, :, t, :].rearrange("p (hc f) b -> p hc f b", hc=2)
                for c in range(2):
                    sl = slice(c * 256, (c + 1) * 256)
                    nc.scalar.activation(r_s[:, sl], ps_r[:, sl], AF.Sigmoid)
                    nc.scalar.activation(z_s[:, sl], ps_z[:, sl], AF.Sigmoid)
                    nc.vector.tensor_tensor(nt[:, sl], ps_ngh[:, sl], r_s[:, sl],
                                            op=ALU.mult)
                    nc.vector.tensor_tensor(nt[:, sl], nt[:, sl], ps_ngi[:, sl],
                                            op=ALU.add)
                    nc.scalar.activation(n_s[:, sl], nt[:, sl], AF.Tanh)
                    warm(tp, 3 - 2 * c, nt[:, c * 256:c * 256 + 128])
                    nc.vector.tensor_tensor(t4[:, sl], hprev[:, sl], n_s[:, sl],
                                            op=ALU.subtract)
                    nc.vector.tensor_tensor(t4[:, sl], t4[:, sl], z_s[:, sl],
                                            op=ALU.mult)
                    warm(tp, 3 - 2 * c, t4[:, c * 256:c * 256 + 128])
                    nc.vector.tensor_tensor(hnew[:, sl], n_s[:, sl], t4[:, sl],
                                            op=ALU.add)
                    transp_half(hnew, tp, c)
                    copy_half(eo_view[:, :, 2 * c:2 * c + 2, :], tp, c)

        # =======================================================
        # Phase 2: encC = enc_out @ combW_app   (+ load decoder weights)
        # =======================================================
        decw2 = ctx.enter_context(tc.tile_pool(name="decw2", bufs=1))
        decWihT = decw2.tile([128, KH, G3], dt.bfloat16, tag="decWihT")
        nc.sync.dma_start(decWihT[:], decWihT_d.ap().rearrange("(k p) n -> p k n", p=128))
        encC = decw2.tile([128, MT, H], dt.bfloat16, tag="encC")
        lg_all = decw2.tile([BL, L, T], dt.float32, tag="lg_all")
        hTt = decw2.tile([128, KH, BL], dt.bfloat16, tag="hTt")
        oTt = decw2.tile([128, KH, BL], dt.bfloat16, tag="oTt")
        with tc.tile_pool(name="ccw", bufs=1) as ccw, \
             tc.tile_pool(name="ccps", bufs=4, space="PSUM") as ccps:
            combWappT = ccw.tile([128, KH, H], dt.bfloat16, tag="combWappT")
            nc.sync.dma_start(combWappT[:],
                              combWappT_d.ap().rearrange("(k p) n -> p k n", p=128))
            for m in range(MT):
                for nch in range(2):
                    ps = ccps.tile([128, 512], dt.float32, tag="cc")
                    for k in range(KH):
                        nc.tensor.matmul(
                            ps[:], enc_outT[:, k, 2 * m:2 * m + 2, :],
                            combWappT[:, k, nch * 512:(nch + 1) * 512],
                            start=(k == 0), stop=(k == KH - 1))
                    nc.vector.tensor_copy(encC[:, m, nch * 512:(nch + 1) * 512], ps[:])
        nc.vector.tensor_copy(hTt[:], enc_outT[:, :, 24, :])

        # =======================================================
        # Phase 3: decoder (25 steps)
        # =======================================================
        with tc.tile_pool(name="dgps", bufs=1, space="PSUM") as dgps, \
             tc.tile_pool(name="dops", bufs=1, space="PSUM") as dops, \
             tc.tile_pool(name="tpp2", bufs=1, space="PSUM") as tpp2, \
             tc.tile_pool(name="mscp", bufs=1, space="PSUM") as mscp, \
             tc.tile_pool(name="lgps", bufs=1, space="PSUM") as lgps, \
             tc.tile_pool(name="dwork", bufs=2) as dwork:
            oh_prev = None
            for t in range(L):
                # ---- argmax token of step t-1 -> ohT, then scores, then gh:
                # the softmax chain runs on ACT/DVE while gh fills the PE,
                # and the einsum follows the gh burst on a warm PE. ----
                if t > 0:
                    tp0 = tpp2.tile([128, 4, 2, BL], dt.float32, tag="tp")
                    nc.tensor.transpose(tp0[:, 0, :, :], oh_prev[:], ident[0:BL, :])
                    ohT = dwork.tile([128, BL], dt.bfloat16, tag="ohT")
                    nc.vector.tensor_copy(ohT[:], tp0[:, 0, 0, :])
                # ---- attention scores -> misc[0:64, 128:153] ----
                misc = mscp.tile([128, 512], dt.float32, tag="misc")
                sc = misc[0:BL, 128:128 + L]
                if t > 0:
                    nc.tensor.matmul(sc, ohT[:], EA[:], start=True, stop=False)
                for ki, k in enumerate(KORD):
                    nc.tensor.matmul(sc, hTt[:, k, :], attnWT[:, k, :],
                                     start=(t == 0 and ki == 0), stop=False)
                nc.tensor.matmul(sc, ones_sb[0:1, 0:BL],
                                 attnb_r[:] if t > 0 else attnb0_r[:],
                                 start=False, stop=True)
                # ---- gh matmuls ----
                ps_r = dgps.tile([128, 512], dt.float32, tag="r")
                ps_z = dgps.tile([128, 512], dt.float32, tag="z")
                ps_ngh = dgps.tile([128, 512], dt.float32, tag="ngh")
                ps_ngi = dgps.tile([128, 512], dt.float32, tag="ngi")
                for g, bank in ((0, ps_r), (1, ps_z), (2, ps_ngh)):
                    for ki, k in enumerate(KORD):
                        for hc in range(2):
                            co = g * H + hc * 512
                            nc.tensor.matmul(bank[hc * 64:(hc + 1) * 64, :],
                                             hTt[:, k, :], decWhhT[:, k, co:co + 512],
                                             start=(ki == 0), stop=False)
                for hc in range(2):
                    nc.tensor.matmul(ps_ngh[hc * 64:(hc + 1) * 64, :],
                                     ones_sb[0:1, 0:BL],
                                     dbhn_r[0:1, hc * 512:hc * 512 + 512],
                                     start=False, stop=True)
                # ---- softmax over scores (no max shift: scores are small) ----
                aw = dwork.tile([BL, L], dt.float32, tag="aw")
                sume = dwork.tile([BL, 1], dt.float32, tag="sume")
                nc.scalar.activation(aw[:], sc, AF.Exp, accum_out=sume[:])
                rs = dwork.tile([BL, 1], dt.float32, tag="rs")
                nc.vector.reciprocal(rs[:], sume[:])
                rs2 = dwork.tile([128, 1], dt.float32, tag="rs2")
                nc.vector.tensor_copy(rs2[0:BL, :], rs[:])
                nc.vector.tensor_copy(rs2[BL:128, :], rs[:])
                # awn[q, p] = aw[b, 2p + (q>=64)] / sum  (l=25 slot zero)
                awn = dwork.tile([128, MT], dt.float32, tag="awn")
                nc.vector.tensor_copy(awn[0:BL, :], aw[:, 0:25:2])
                nc.vector.tensor_copy(awn[BL:128, 0:12], aw[:, 1:25:2])
                nc.vector.memset(awn[BL:128, 12:13], 0.0)
                nc.vector.tensor_scalar(awn[:], awn[:], rs2[:], None, op0=ALU.mult)
                dgs = dwork.tile([128, MT, BL], dt.bfloat16, tag="dgs", bufs=1)
                nc.vector.tensor_tensor(dgs[:, 0:7, :], IstkB[:, 0:7, :],
                                        awn[:, 0:7].broadcast_to((128, 7, BL)),
                                        op=ALU.mult)
                nc.vector.tensor_tensor(dgs[:, 7:MT, :], IstkB[:, 7:MT, :],
                                        awn[:, 7:MT].broadcast_to((128, 6, BL)),
                                        op=ALU.mult)
                # ---- o = emb-part(via EC) + einsum(aw, encC) + combb ----
                ps_o = dops.tile([128, 512], dt.float32, tag="o")
                if t > 0:
                    for hc in range(2):
                        nc.tensor.matmul(ps_o[hc * 64:(hc + 1) * 64, :], ohT[:],
                                         EC[:, hc * 512:(hc + 1) * 512],
                                         start=True, stop=False)
                for hc in range(2):
                    nc.tensor.matmul(ps_o[hc * 64:(hc + 1) * 64, :],
                                     ones_sb[0:1, 0:BL],
                                     combb_r[0:1, hc * 512:hc * 512 + 512]
                                     if t > 0 else
                                     combb0_r[0:1, hc * 512:hc * 512 + 512],
                                     start=(t == 0), stop=False)
                for p in range(MT):
                    for hc in range(2):
                        nc.tensor.matmul(ps_o[hc * 64:(hc + 1) * 64, :],
                                         dgs[:, p, :],
                                         encC[:, p, hc * 512:(hc + 1) * 512],
                                         start=False, stop=(p == MT - 1))
                obf = dwork.tile([128, 512], dt.float32, tag="obf")
                nc.scalar.activation(obf[:], ps_o[:], AF.Relu, scale=S2_SCALE)
                # ---- oT ----
                tp = tpp2.tile([128, 4, 2, BL], dt.float32, tag="tp")
                for c in range(2):
                    transp_half(obf, tp, c)
                nc.vector.tensor_copy(
                    oTt[:].rearrange("p (hc f) b -> p hc f b", hc=2),
                    tp[:].rearrange("p f hc b -> p hc f b"))
                # ---- gi matmuls from oT; bank order r, ngi, z ----
                for g, bank in ((0, ps_r), (2, ps_ngi), (1, ps_z)):
                    for ki, k in enumerate(KORD):
                        for hc in range(2):
                            co = g * H + hc * 512
                            nc.tensor.matmul(bank[hc * 64:(hc + 1) * 64, :],
                                             oTt[:, k, :], decWihT[:, k, co:co + 512],
                                             start=(g == 2 and ki == 0), stop=False)
                    for hc in range(2):
                        co = g * H + hc * 512
                        nc.tensor.matmul(bank[hc * 64:(hc + 1) * 64, :],
                                         ones_sb[0:1, 0:BL],
                                         dgib_r[0:1, co:co + 512],
                                         start=False, stop=True)
                # ---- gates (sigma via tanh), chunked halves ----
                hprev = h_tiles[(L + t) % 2]
                hnew = h_tiles[(L + t + 1) % 2]
                r_s = dwork.tile([128, 512], dt.bfloat16, tag="r_s")
                z_s = dwork.tile([128, 512], dt.bfloat16, tag="z_s")
                nt = dwork.tile([128, 512], dt.float32, tag="nt", bufs=1)
                n_s = dwork.tile([128, 512], dt.float32, tag="n_s", bufs=1)
                t4 = dwork.tile([128, 512], dt.float32, tag="t4", bufs=1)
                tp2 = tpp2.tile([128, 4, 2, BL], dt.float32, tag="tp")
                lg = lgps.tile([BL, T], dt.float32, tag="lg")
                hT_view = hTt[:].rearrange("p (hc f) b -> p hc f b", hc=2)
                for c in range(2):
                    sl = slice(c * 256, (c + 1) * 256)
                    nc.scalar.activation(r_s[:, sl], ps_r[:, sl], AF.Tanh, scale=0.5)
                    nc.vector.tensor_scalar(r_s[:, sl], r_s[:, sl], 0.5, 0.5,
                                            op0=ALU.mult, op1=ALU.add)
                    nc.scalar.activation(z_s[:, sl], ps_z[:, sl], AF.Tanh, scale=0.5)
                    nc.vector.tensor_scalar(z_s[:, sl], z_s[:, sl], 0.5, 0.5,
                                            op0=ALU.mult, op1=ALU.add)
                    nc.vector.tensor_tensor(nt[:, sl], ps_ngh[:, sl], r_s[:, sl],
                                            op=ALU.mult)
                    nc.vector.tensor_tensor(nt[:, sl], nt[:, sl], ps_ngi[:, sl],
                                            op=ALU.add)
                    nc.scalar.activation(n_s[:, sl], nt[:, sl], AF.Tanh)
                    warm(tp2, 3 - 2 * c, nt[:, c * 256:c * 256 + 128])
                    nc.vector.tensor_tensor(t4[:, sl], hprev[:, sl], n_s[:, sl],
                                            op=ALU.subtract)
                    nc.vector.tensor_tensor(t4[:, sl], t4[:, sl], z_s[:, sl],
                                            op=ALU.mult)
                    warm(tp2, 3 - 2 * c, t4[:, c * 256:c * 256 + 128])
                    nc.vector.tensor_tensor(hnew[:, sl], n_s[:, sl], t4[:, sl],
                                            op=ALU.add)
                    transp_half(hnew, tp2, c)
                    copy_half(hT_view[:, :, 2 * c:2 * c + 2, :], tp2, c)
                    # logits for the k-tiles this half provides
                    for ki, k in enumerate(KORD[4 * c:4 * c + 4]):
                        nc.tensor.matmul(lg[:], hTt[:, k, :], outWTs[:, k, :],
                                         start=(c == 0 and ki == 0), stop=False)
                nc.tensor.matmul(lg[:], ones_sb[0:1, 0:BL], lgb_r[:],
                                 start=False, stop=True)
                nc.scalar.copy(lg_all[:, t, :], lg[:])
                ex = dwork.tile([BL, T], dt.float32, tag="ex")
                nc.scalar.activation(ex[:], lg[:], AF.Exp,
                                     accum_out=se_all[:, t:t + 1])
                warm(tp2, 1, ex[:])
                # ---- argmax onehot (transposed next iteration) ----
                if t < L - 1:
                    mx2 = dwork.tile([BL, 1], dt.float32, tag="mx2")
                    nc.vector.tensor_reduce(mx2[:], lg[:], axis=AX.X, op=ALU.max)
                    oh_prev = dwork.tile([BL, T], dt.float32, tag="oh")
                    nc.vector.tensor_scalar(oh_prev[:], lg[:], mx2[:], None,
                                            op0=ALU.is_equal)

        # =======================================================
        # Phase 4: log-softmax tail
        # =======================================================
        with tc.tile_pool(name="tail", bufs=2) as tail:
            lse = tail.tile([BL, L], dt.float32, tag="lse", bufs=1)
            nc.scalar.activation(lse[:], se_all[:], AF.Ln)
            for t in range(L):
                lout = tail.tile([BL, T], dt.float32, tag="lout")
                nc.vector.tensor_scalar(lout[:], lg_all[:, t, :], lse[:, t:t + 1],
                                        None, op0=ALU.subtract)
                nc.sync.dma_start(
                    out_d.ap().rearrange("(b l) c -> b l c", l=L)[:, t, :], lout[:])
    nc.finalize()
    return nc


S2_SCALE = 1.0  # patched at build time (bn2 scale); module-level for closure use


def kernel(**inputs):
    global S2_SCALE
    import concourse.bass_utils as bass_utils

    tokens = np.asarray(inputs["tokens"])
    w2v = np.asarray(inputs["w2v"], np.float32)
    bn1 = np.asarray(inputs["bn1"], np.float32)
    bn2 = np.asarray(inputs["bn2"], np.float32)
    s1 = float(bn1[0] / np.sqrt(bn1[3] + BN_EPS))
    t1 = float(bn1[1] - bn1[2] * s1)
    s2 = float(bn2[0] / np.sqrt(bn2[3] + BN_EPS))
    t2 = float(bn2[1] - bn2[2] * s2)
    S2_SCALE = s2

    f32 = lambda k: np.asarray(inputs[k], np.float32)
    bft = lambda a: np.ascontiguousarray(np.asarray(a, np.float32).T).astype(BF16)
    enc_bih, enc_bhh = f32("enc_bih"), f32("enc_bhh")
    dec_bih, dec_bhh = f32("dec_bih"), f32("dec_bhh")
    egib = np.concatenate([enc_bih[:H] + enc_bhh[:H], enc_bih[H:2 * H] + enc_bhh[H:2 * H],
                           enc_bih[2 * H:]])
    dgib = np.concatenate([dec_bih[:H] + dec_bhh[:H], dec_bih[H:2 * H] + dec_bhh[H:2 * H],
                           dec_bih[2 * H:]])[None, :]
    out_W = f32("out_W")
    outWTs = np.ascontiguousarray((s1 * out_W).T).astype(BF16)
    lgb = (f32("out_b") + t1 * out_W.sum(axis=1))[None, :]
    combb = (f32("comb_b") + t2 / s2)[None, :]
    comb_W = f32("comb_W")

    # encoder Wih with bias folded as last row
    encWihT = np.zeros((301, G3), np.float32)
    encWihT[:300] = f32("enc_Wih").T
    encWihT[300] = egib

    # dec_emb rows 0..127 renormed (host); fold emb@attnW_emb / emb@combW_emb
    em = f32("dec_emb")[:128]
    emn = np.linalg.norm(em, axis=1, keepdims=True)
    embf = em * np.minimum(1.0, MAXN2 / (emn + 1e-7))
    attn_W = f32("attn_W")
    EA = embf @ attn_W[:, :D].T                       # (128, L)
    EC = embf @ comb_W[:, :D].T                       # (128, H)
    # SOS embedding renormed -> step-0 bias rows
    sos = f32("dec_emb")[T]
    sos = sos * min(1.0, MAXN2 / (np.linalg.norm(sos) + 1e-7))
    attnb = f32("attn_b")[None, :]
    attnb0 = attnb + (sos @ attn_W[:, :D].T)[None, :]
    combb0 = combb + (sos @ comb_W[:, :D].T)[None, :]

    ident = np.eye(128, dtype=np.float32)
    istk = np.zeros((128, BL), np.float32)
    istk[np.arange(128), np.arange(128) % BL] = 1.0
    istkb = np.tile(istk, (1, MT))

    common = {
        "encWihT": encWihT.astype(BF16), "encWhhT": bft(inputs["enc_Whh"]),
        "decWihT": bft(inputs["dec_Wih"]), "decWhhT": bft(inputs["dec_Whh"]),
        "combWappT": np.ascontiguousarray(comb_W[:, D:].T).astype(BF16),
        "outWTs": outWTs,
        "attnWT": np.ascontiguousarray(attn_W[:, D:].T).astype(BF16),
        "EA": np.ascontiguousarray(EA).astype(BF16),
        "EC": np.ascontiguousarray(EC).astype(BF16),
        "attnb0": np.ascontiguousarray(attnb0).astype(BF16),
        "combb0": np.ascontiguousarray(combb0).astype(BF16),
        "ident": ident, "istkb": istkb.astype(BF16),
        "ebhn": np.ascontiguousarray(enc_bhh[2 * H:][None, :]).astype(BF16),
        "dgib": np.ascontiguousarray(dgib).astype(BF16),
        "dbhn": np.ascontiguousarray(dec_bhh[2 * H:][None, :]).astype(BF16),
        "combb": np.ascontiguousarray(combb).astype(BF16),
        "attnb": np.ascontiguousarray(attnb).astype(BF16),
        "lgb": np.ascontiguousarray(lgb).astype(BF16),
    }
    in_maps = []
    for c in range(NC):
        tok = tokens[c * BL:(c + 1) * BL].astype(np.int64)        # (64,25)
        xg = w2v[tok]                                             # (64,25,300)
        nrm = np.linalg.norm(xg, axis=-1, keepdims=True)
        xg = xg * np.minimum(1.0, MAXN1 / (nrm + 1e-7))
        xTb = np.zeros((301, L * BL), np.float32)
        xTb[:300] = xg.transpose(2, 1, 0).reshape(E, L * BL)      # col = l*64+b
        xTb[300] = 1.0
        m = dict(common)
        m["xTb"] = xTb.astype(BF16)
        in_maps.append(m)

    nc = build_nc()
    trace = bool(int(os.environ.get("KERNEL_TRACE", "0")))
    res = bass_utils.run_bass_kernel_spmd(nc, in_maps, core_ids=list(range(NC)),
                                          trace=trace)
    if trace and res.exec_time_ns is not None:
        print(f"HW exec time: {res.exec_time_ns} ns", flush=True)
        print("trace:", res.instructions_and_trace[1] if res.instructions_and_trace else None,
              flush=True)
    out = np.concatenate([res.results[c]["out"] for c in range(NC)], axis=0)
    return out.astype(np.float32)


if __name__ == "__main__":
    pass
